# revision 16
# baseline (speedup 1.0000x reference)
"""H2GCN forward pass on 8 Trainium2 NeuronCores (Bass/Tile SPMD kernel).

Strategy (1D row-parallel SpMM; fp16 conv1, fp8-DoubleRow conv2):
  - Nodes are sharded across 8 cores (1024 rows each). Each core receives
    column-slices adjT = adj[rows, :].T ([8192, 1024]) of both normalized
    adjacency matrices in fp16 (conv1) and of the BINARY patterns in fp8
    (conv2). The gcn normalization A = S*Abin*S (S = diag(1/sqrt(deg)))
    is factored out for conv2: Abin is EXACT in fp8 (entries 0/1), the
    per-node s-scales fold into the z quantization (pre-scale) and the
    PSUM->SBUF copies (post-scale), so conv2's adjacency contributes zero
    quantization error while running at fp8 DoubleRow speed (2 fp8 weights
    per PE cell, 2 contraction k-tiles per matmul, ~1.8x fp16 rate).
  - The feature embed is *replicated* (fp16): every core computes the full h
    for all 8192 nodes in node-major layout instead of all-gathering it.
  - conv1 (fp16, full precision path): zT = [A@h; A2@h].T feature-major, RAW.
  - BatchNorm is *algebraically absorbed* into the final projection:
        z_n = z*c + d  with c = gamma*rsqrt(var+eps), d = beta - mean*c
        A@z_n = (A@z)*c + rowsum(A) (x) d
    so conv2 runs on raw z and the final projection applies per-row scaled
    weights plus rank-1 corrections from host-exact rowsums.
  - z is MEAN-CENTERED (host-exact colmeans mu_z via the colsum identity
    mean(A@h) = (colsum(A)/N)@h) before fp8 quantization: the quantization
    error then scales with the per-column FLUCTUATION - exactly what BN
    normalizes by - instead of the column mean, avoiding the ~30x error
    amplification of the near-constant A2@h columns. The centering is
    algebraically exact: A@z = A@(z-mu) + rowsum(A) (x) mu, and the rank-1
    correction folds into the existing BN-absorption vector (d' = c*mu + d)
    at ZERO device cost.
  - Centered z is transposed to node-major, quantized twice (s1- and
    s2-pre-scaled fp8), and AllGathered in two feature-halves into
    addr_space="Shared" DRAM (fast HBM-HBM collective path), with
    partition-swizzled layout so gather-in and SBUF reload DMAs are
    contiguous 2KB-per-partition runs. The reloads run on the sync queue so
    they are NOT serialized behind the BN AllReduce on the gpsimd/CC queue.
    z1's gather hides under conv1's second half, z2's under conv2's first
    m-pass. BN statistics use a tiny AllReduce, off the critical path.
"""

import numpy as np

import concourse.bass as bass
import concourse.mybir as mybir
import concourse.tile as tile
from concourse import bacc
from concourse.bass_utils import run_bass_kernel_spmd
from concourse.masks import make_identity

P = 128
NCORES = 8
BN_EPS = 1e-5

F8 = mybir.dt.float8e4
F16 = mybir.dt.float16
F32 = mybir.dt.float32
DR = mybir.MatmulPerfMode.DoubleRow

FULL_CFG = dict(NT=8192, R=1024)
IN_CH = 512   # input features
H = 256       # hidden
H2 = 512      # 2*H (BN width)
O = 64        # output features
F = 7 * H     # 1792, JK concat width


def _nchunks(R):
    """Split the per-core node free-dim R into <=512 chunks (PSUM bank width)."""
    out = []
    s = 0
    while s < R:
        w = min(512, R - s)
        out.append((s, w))
        s += w
    return out


def build_program(NT, R, scales):
    """Build the SPMD Bass program. NT = total nodes, R = rows per core.
    scales = (SZ1, SZ2): power-of-2 exponents for the fp8 z quantization."""
    SZ1, SZ2 = scales
    KT = NT // P           # node k-tiles (contraction tiles)
    K2 = KT // 2           # DoubleRow k-tile pairs
    RT = R // P            # per-core node tiles (free-dim tiles / transposes)
    NCH = _nchunks(R)
    NC2 = len(NCH)
    HM = H // P            # 2  (hidden chunks)
    H2M = H2 // P          # 4
    FM = F // P            # 14
    INK = IN_CH // P       # 4

    nc = bacc.Bacc("TRN2", target_bir_lowering=False, debug=False,
                   num_devices=NCORES)

    # --- I/O -------------------------------------------------------------
    xTf = nc.dram_tensor("xTf", [IN_CH, NT], F16, kind="ExternalInput")
    xT = nc.dram_tensor("xT", [IN_CH, R], F16, kind="ExternalInput")
    abT = nc.dram_tensor("abT", [NT, R], F8, kind="ExternalInput")
    abT2 = nc.dram_tensor("abT2", [NT, R], F8, kind="ExternalInput")
    wTe = nc.dram_tensor("wTe", [IN_CH, H], F16, kind="ExternalInput")
    be = nc.dram_tensor("be", [P, HM], F32, kind="ExternalInput")
    bebc = nc.dram_tensor("bebc", [P, H], F16, kind="ExternalInput")
    wTf = nc.dram_tensor("wTf", [F, O], F16, kind="ExternalInput")
    bff = nc.dram_tensor("bff", [O, 1], F32, kind="ExternalInput")
    gam = nc.dram_tensor("gam", [P, H2M], F32, kind="ExternalInput")
    bet = nc.dram_tensor("bet", [P, H2M], F32, kind="ExternalInput")
    nmuz = nc.dram_tensor("nmuz", [P, H2M], F32, kind="ExternalInput")
    muz = nc.dram_tensor("muz", [P, H2M], F32, kind="ExternalInput")
    s1q = nc.dram_tensor("s1q", [P, RT], F32, kind="ExternalInput")
    s2q = nc.dram_tensor("s2q", [P, RT], F32, kind="ExternalInput")
    s1f = nc.dram_tensor("s1f", [P, KT], F32, kind="ExternalInput")
    s2f = nc.dram_tensor("s2f", [P, KT], F32, kind="ExternalInput")
    s1c = nc.dram_tensor("s1c", [P, R], F16, kind="ExternalInput")
    s2c = nc.dram_tensor("s2c", [P, R], F16, kind="ExternalInput")
    rsA = nc.dram_tensor("rsA", [O, R], F16, kind="ExternalInput")
    rsA2 = nc.dram_tensor("rsA2", [O, R], F16, kind="ExternalInput")
    out = nc.dram_tensor("out", [R, O], F32, kind="ExternalOutput")

    rg = [list(range(NCORES))]

    with tile.TileContext(nc) as tc:
        with (
            tc.tile_pool(name="const", bufs=1) as const,
            tc.tile_pool(name="feat", bufs=1) as feat,
            tc.tile_pool(name="tmp", bufs=2) as tmp,
            tc.tile_pool(name="stream", bufs=5) as stream,
            tc.tile_pool(name="ps", bufs=1, space="PSUM") as ps,
            tc.tile_pool(name="dram", bufs=1, space="DRAM") as dram,
        ):
            # --- constants / weights (embed-critical ones first) --------
            wTe_sb = const.tile([P, INK, H], F16, name="wTe_sb")
            nc.sync.dma_start(wTe_sb[:], wTe.ap().rearrange("(k p) m -> p k m", p=P))
            bebc_sb = const.tile([P, H], F16, name="bebc_sb")
            nc.sync.dma_start(bebc_sb[:], bebc.ap())
            xT_sb = const.tile([P, INK, R], F16, name="xT_sb")
            nc.sync.dma_start(xT_sb[:], xT.ap().rearrange("(k p) n -> p k n", p=P))
            be_sb = const.tile([P, HM], F32, name="be_sb")
            nc.sync.dma_start(be_sb[:], be.ap())
            id16 = const.tile([P, P], F16, name="id16")
            make_identity(nc, id16)
            id32 = const.tile([P, P], F32, name="id32")
            make_identity(nc, id32)
            nmuz_sb = const.tile([P, H2M], F32, name="nmuz_sb")
            nc.sync.dma_start(nmuz_sb[:], nmuz.ap())
            s1q_sb = const.tile([P, RT], F32, name="s1q_sb")
            nc.sync.dma_start(s1q_sb[:], s1q.ap())
            s2q_sb = const.tile([P, RT], F32, name="s2q_sb")
            nc.sync.dma_start(s2q_sb[:], s2q.ap())
            s1f_sb = const.tile([P, KT], F32, name="s1f_sb")
            nc.sync.dma_start(s1f_sb[:], s1f.ap())
            s2f_sb = const.tile([P, KT], F32, name="s2f_sb")
            nc.sync.dma_start(s2f_sb[:], s2f.ap())
            s1c_sb = const.tile([P, R], F16, name="s1c_sb")
            nc.sync.dma_start(s1c_sb[:], s1c.ap())
            s2c_sb = const.tile([P, R], F16, name="s2c_sb")
            nc.sync.dma_start(s2c_sb[:], s2c.ap())

            # full x.T, staged through a double-buffer (32 KB instead of
            # 64 KB resident) so the embed starts early and SBUF stays small
            xTf_t = xTf.ap().rearrange("(k p) n -> p k n", p=P)
            XG = 2048
            KG = XG // P

            # --- phase B1: replicated full embed, node-major ------------
            # hs_a[node, feat] = s_a[node] * relu(x @ w_embed.T + b), the
            # row-scaled h needed by the factored conv1 (A = S Abin S).
            # relu(s*x) = s*relu(x) for s>=0, so the s-scale rides the relu.
            hs1_sb = feat.tile([P, KT, H], F16, name="hs1_sb", tag="kxnA")
            hs2_sb = feat.tile([P, KT, H], F16, name="hs2_sb")
            xg_sb = None
            for k in range(KT):
                if k % KG == 0:
                    g = k * P
                    xg_sb = feat.tile([P, INK, XG], F16, name=f"xg_{k}",
                                      tag="xg", bufs=2)
                    nc.sync.dma_start(xg_sb[:], xTf_t[:, :, g:g + XG])
                hps = ps.tile([P, H], F32, name=f"hps_{k}", tag=f"acc{k % 8}")
                for t in range(INK):
                    nc.tensor.matmul(
                        hps[:],
                        lhsT=xg_sb[:, t, (k % KG) * P:(k % KG + 1) * P],
                        rhs=wTe_sb[:, t, :],
                        start=(t == 0), stop=(t == INK - 1),
                    )
                nc.vector.tensor_tensor(
                    out=hps[:], in0=hps[:], in1=bebc_sb[:],
                    op=mybir.AluOpType.add)
                nc.scalar.activation(
                    hs1_sb[:, k, :], hps[:],
                    mybir.ActivationFunctionType.Relu,
                    scale=s1f_sb[:, k:k + 1])
                nc.vector.tensor_scalar(
                    out=hs2_sb[:, k, :], in0=hps[:],
                    scalar1=0.0, scalar2=s2f_sb[:, k:k + 1],
                    op0=mybir.AluOpType.max, op1=mybir.AluOpType.mult)

            # --- phase B2: local embed, feature-major (for the JK concat)
            hT_sb = feat.tile([P, HM, R], F16, name="hT_sb")
            for m in range(HM):
                for ci, (cs, cw) in enumerate(NCH):
                    eps_t = ps.tile([P, 512], F32, name=f"eps_{m}_{ci}",
                                    tag=f"acc{(m * NC2 + ci) % 8}")
                    for t in range(INK):
                        nc.tensor.matmul(
                            eps_t[:, :cw],
                            lhsT=wTe_sb[:, t, m * P:(m + 1) * P],
                            rhs=xT_sb[:, t, cs:cs + cw],
                            start=(t == 0), stop=(t == INK - 1),
                        )
                    nc.scalar.activation(
                        hT_sb[:, m, cs:cs + cw], eps_t[:, :cw],
                        mybir.ActivationFunctionType.Relu,
                        bias=be_sb[:, m:m + 1],
                    )

            # --- phase D: conv1 (factored: fp16 hs x fp8 binary Abin) ---
            # z = S (Abin @ (S h)): the streamed rhs is the EXACT binary
            # pattern in fp8 (half the bytes of fp16 normalized A, zero
            # quantization error); the output s-post-scale folds into the
            # PSUM->SBUF copy. One adjacency half at a time; each half's
            # centered transposes + dual fp8 quantize + AllGather overlap
            # the next compute.
            zT_sb = feat.tile([P, H2M, R], F16, name="zT_sb")
            zag_out = [None, None]
            for half, (src, hs_sb, sc_sb) in (
                    (0, (abT, None, None)), (1, (abT2, None, None))):
                hs_sb = hs1_sb if half == 0 else hs2_sb
                sc_sb = s1c_sb if half == 0 else s2c_sb
                zps = {}
                for m in range(HM):
                    for ci in range(NC2):
                        zps[(m, ci)] = ps.tile(
                            [P, 512], F32, name=f"zps_{half}_{m}_{ci}",
                            tag=f"acc{(half * 4 + m * NC2 + ci) % 8}")
                for k in range(KT):
                    at = stream.tile([P, R], F8, name=f"c1_{half}_{k}", tag="adj")
                    nc.sync.dma_start(at[:], src[k * P:(k + 1) * P, :])
                    for m in range(HM):
                        for ci, (cs, cw) in enumerate(NCH):
                            nc.tensor.matmul(
                                zps[(m, ci)][:, :cw],
                                lhsT=hs_sb[:, k, m * P:(m + 1) * P],
                                rhs=at[:, cs:cs + cw],
                                start=(k == 0), stop=(k == KT - 1),
                            )
                # raw z = s_out * PSUM (vector) + centered z (scalar)
                zcT = tmp.tile([P, HM, R], F16, name="zcT", tag="zcT", bufs=1)
                for m in range(HM):
                    for ci, (cs, cw) in enumerate(NCH):
                        nc.vector.tensor_tensor(
                            out=zT_sb[:, half * HM + m, cs:cs + cw],
                            in0=zps[(m, ci)][:, :cw],
                            in1=sc_sb[:, cs:cs + cw],
                            op=mybir.AluOpType.mult)
                        nc.scalar.activation(
                            zcT[:, m, cs:cs + cw],
                            zT_sb[:, half * HM + m, cs:cs + cw],
                            mybir.ActivationFunctionType.Identity,
                            bias=nmuz_sb[:, half * HM + m:half * HM + m + 1])

                # transpose centered z to node-major and quantize twice
                # (s1- and s2-pre-scaled fp8), then AllGather both
                zs_nm = tmp.tile([P, RT, 2, H], F8, name="znm", tag="znm", bufs=1)
                for fi in range(HM):
                    for nt in range(RT):
                        tps = ps.tile(
                            [P, P], F16, name=f"ztp_{half}_{fi}_{nt}",
                            tag=f"acc{(fi * RT + nt) % 4 + half * 4}")
                        nc.tensor.transpose(
                            tps[:], zcT[:, fi, nt * P:(nt + 1) * P],
                            id16[:])
                        nc.scalar.mul(
                            zs_nm[:, nt, 0, fi * P:(fi + 1) * P], tps[:],
                            s1q_sb[:, nt:nt + 1])
                        nc.scalar.mul(
                            zs_nm[:, nt, 1, fi * P:(fi + 1) * P], tps[:],
                            s2q_sb[:, nt:nt + 1])
                zin = dram.tile([P, RT, 2, H], F8, name=f"zag_in_{half}")
                nc.gpsimd.dma_start(zin[:], zs_nm[:])
                zout = dram.tile([NCORES, P, RT, 2, H], F8,
                                 name=f"zag_out_{half}", addr_space="Shared")
                nc.gpsimd.collective_compute(
                    "AllGather", mybir.AluOpType.bypass, replica_groups=rg,
                    ins=[zin.opt()], outs=[zout.opt()],
                )
                zag_out[half] = zout

            # --- phase E: BN stats + AllReduce (off critical path) ------
            stat_sb = tmp.tile([P, 2 * H2M], F32, name="stat_sb", bufs=1)
            for f in range(H2M):
                sq = tmp.tile([P, R], F16, name="sq", tag="zcT", bufs=1)
                nc.scalar.activation(
                    sq[:], zT_sb[:, f, :], mybir.ActivationFunctionType.Copy,
                    accum_out=stat_sb[:, f:f + 1])
                sq2 = tmp.tile([P, R], F16, name="sq2", tag="zcT", bufs=1)
                nc.scalar.activation(
                    sq2[:], zT_sb[:, f, :], mybir.ActivationFunctionType.Square,
                    accum_out=stat_sb[:, H2M + f:H2M + f + 1])
            ar_in = dram.tile([P, 2 * H2M], F32, name="ar_in")
            nc.gpsimd.dma_start(ar_in[:], stat_sb[:])
            ar_out = dram.tile([P, 2 * H2M], F32, name="ar_out")
            nc.gpsimd.collective_compute(
                "AllReduce", mybir.AluOpType.add, replica_groups=rg,
                ins=[ar_in.opt()], outs=[ar_out.opt()],
            )
            stat_g = tmp.tile([P, 2 * H2M], F32, name="stat_g", bufs=1)
            nc.gpsimd.dma_start(stat_g[:], ar_out[:])

            # BN coefficients c, d (feature-major [128, 4], fp32)
            gam_sb = const.tile([P, H2M], F32, name="gam_sb")
            nc.sync.dma_start(gam_sb[:], gam.ap())
            bet_sb = const.tile([P, H2M], F32, name="bet_sb")
            nc.sync.dma_start(bet_sb[:], bet.ap())
            muz_sb = const.tile([P, H2M], F32, name="muz_sb")
            nc.sync.dma_start(muz_sb[:], muz.ap())
            cmean = tmp.tile([P, H2M], F32, name="cmean", bufs=1)
            nc.scalar.mul(cmean[:], stat_g[:, 0:H2M], 1.0 / NT)
            cvar = tmp.tile([P, H2M], F32, name="cvar", bufs=1)
            nc.scalar.mul(cvar[:], stat_g[:, H2M:2 * H2M], 1.0 / NT)
            msq = tmp.tile([P, H2M], F32, name="msq", bufs=1)
            nc.vector.tensor_mul(out=msq[:], in0=cmean[:], in1=cmean[:])
            nc.vector.tensor_tensor(
                out=cvar[:], in0=cvar[:], in1=msq[:],
                op=mybir.AluOpType.subtract)
            eps_sb = tmp.tile([P, 1], F32, name="eps_sb", bufs=1)
            nc.vector.memset(eps_sb[:], BN_EPS)
            cstd = tmp.tile([P, H2M], F32, name="cstd", bufs=1)
            nc.scalar.activation(
                cstd[:], cvar[:], mybir.ActivationFunctionType.Sqrt,
                bias=eps_sb[:])
            crstd = tmp.tile([P, H2M], F32, name="crstd", bufs=1)
            nc.vector.reciprocal(crstd[:], cstd[:])
            c_t = tmp.tile([P, H2M], F32, name="c_t", bufs=1)
            nc.vector.tensor_mul(out=c_t[:], in0=crstd[:], in1=gam_sb[:])
            d_t = tmp.tile([P, H2M], F32, name="d_t", bufs=1)
            nc.vector.tensor_mul(out=d_t[:], in0=cmean[:], in1=c_t[:])
            nc.vector.tensor_tensor(
                out=d_t[:], in0=bet_sb[:], in1=d_t[:],
                op=mybir.AluOpType.subtract)
            d16 = tmp.tile([P, H2M], F16, name="d16", bufs=1)
            nc.vector.tensor_copy(out=d16[:], in_=d_t[:])
            # d' = c * mu_z + d  (U-block correction incl. the z centering)
            dp_t = tmp.tile([P, H2M], F32, name="dp_t", bufs=1)
            nc.vector.tensor_mul(out=dp_t[:], in0=c_t[:], in1=muz_sb[:])
            nc.vector.tensor_add(out=dp_t[:], in0=dp_t[:], in1=d_t[:])
            d16p = tmp.tile([P, H2M], F16, name="d16p", bufs=1)
            nc.vector.tensor_copy(out=d16p[:], in_=dp_t[:])

            # --- phase F: conv2 on centered z (fp8 DoubleRow), two ------
            # m-half passes; pass 0 (z features 0:256, from z1) only needs
            # zag_out[0], so it overlaps z2's AllGather. Each pass streams
            # both binary adjacencies.
            ab_p = abT.ap().rearrange("(k2 ko p) n -> p k2 ko n", ko=2, p=P)
            ab2_p = abT2.ap().rearrange("(k2 ko p) n -> p k2 ko n", ko=2, p=P)
            u_sb = feat.tile([P, 2 * H2M, R], F16, name="u_sb")
            for half in (0, 1):
                # reload on the sync queue: NOT serialized behind the
                # AllReduce on the gpsimd/CC queue. Layout [p][kt][a][f]
                # with kt=(r,nt): fully contiguous 4KB runs per (r,p).
                zfd = feat.tile([P, KT, 2, H], F8, name=f"zfd_{half}",
                                tag=("kxnB" if half == 0 else "kxnA"))
                nc.sync.dma_start(
                    zfd[:].rearrange("p (r nt) a f -> p r nt a f", nt=RT),
                    zag_out[half].rearrange("r p nt a f -> p r nt a f"))
                ups = {}
                for a in (0, 1):
                    for m in range(HM):
                        for ci in range(NC2):
                            ups[(a, m, ci)] = ps.tile(
                                [P, 512], F32, name=f"ups_{half}_{a}_{m}_{ci}",
                                tag=f"acc{(a * 4 + m * NC2 + ci) % 8}")
                for k2 in range(K2):
                    at = stream.tile([P, 2, R], F8, name=f"c2a_{half}_{k2}",
                                     tag="adj")
                    nc.sync.dma_start(at[:], ab_p[:, k2])
                    at2 = stream.tile([P, 2, R], F8, name=f"c2b_{half}_{k2}",
                                      tag="adj")
                    nc.sync.dma_start(at2[:], ab2_p[:, k2])
                    for m in range(HM):
                        for ci, (cs, cw) in enumerate(NCH):
                            nc.tensor.matmul(
                                ups[(0, m, ci)][:, :cw],
                                lhsT=zfd[:, 2 * k2:2 * k2 + 2, 0,
                                         m * P:(m + 1) * P],
                                rhs=at[:, :, cs:cs + cw],
                                start=(k2 == 0), stop=(k2 == K2 - 1),
                                perf_mode=DR,
                            )
                            nc.tensor.matmul(
                                ups[(1, m, ci)][:, :cw],
                                lhsT=zfd[:, 2 * k2:2 * k2 + 2, 1,
                                         m * P:(m + 1) * P],
                                rhs=at2[:, :, cs:cs + cw],
                                start=(k2 == 0), stop=(k2 == K2 - 1),
                                perf_mode=DR,
                            )
                # u feature layout: chunks 0..3 = U1 (A@zc), 4..7 = U2
                # (A2@zc); this pass produces z-feature chunks {half*2,
                # half*2+1} of each. Post-scale s_a * 2^-SZa (DVE).
                for a, su in ((0, s1c_sb), (1, s2c_sb)):
                    for m in range(HM):
                        for ci, (cs, cw) in enumerate(NCH):
                            nc.vector.tensor_tensor(
                                out=u_sb[:, a * H2M + half * HM + m, cs:cs + cw],
                                in0=ups[(a, m, ci)][:, :cw],
                                in1=su[:, cs:cs + cw],
                                op=mybir.AluOpType.mult)

            # --- phase G: final projection with absorbed BN -------------
            wTf_sb = const.tile([P, FM, O], F16, name="wTf_sb")
            nc.sync.dma_start(wTf_sb[:], wTf.ap().rearrange("(k p) m -> p k m", p=P))
            bff_sb = const.tile([O, 1], F32, name="bff_sb")
            nc.sync.dma_start(bff_sb[:], bff.ap())
            rsA_sb = const.tile([O, R], F16, name="rsA_sb")
            nc.sync.dma_start(rsA_sb[:], rsA.ap())
            rsA2_sb = const.tile([O, R], F16, name="rsA2_sb")
            nc.sync.dma_start(rsA2_sb[:], rsA2.ap())

            # s_0 = W_zn @ d ; s_j = W_Uj @ d' (j=1,2), from UNSCALED wTf
            s_cols = tmp.tile([O, 3], F32, name="s_cols", bufs=1)
            for j, (base, dv) in enumerate((
                    (HM, d16), (HM + H2M, d16p), (HM + 2 * H2M, d16p))):
                sps = ps.tile([O, 1], F32, name=f"sps_{j}", tag=f"acc{j}")
                for t in range(H2M):
                    nc.tensor.matmul(
                        sps[:], lhsT=wTf_sb[:, base + t, :],
                        rhs=dv[:, t:t + 1],
                        start=(t == 0), stop=(t == H2M - 1))
                nc.vector.tensor_copy(out=s_cols[:, j:j + 1], in_=sps[:])
            s0b = tmp.tile([O, 1], F32, name="s0b", bufs=1)
            nc.vector.tensor_add(out=s0b[:], in0=s_cols[:, 0:1], in1=bff_sb[:])

            # scale wTf rows in place: z_n block by c, U blocks by
            # c * 2^-SZa (u_sb is stored 2^SZa-scaled; descale folds here)
            cu1_t = tmp.tile([P, H2M], F32, name="cu1_t", bufs=1)
            nc.scalar.mul(cu1_t[:], c_t[:], float(2.0 ** -SZ1))
            cu2_t = tmp.tile([P, H2M], F32, name="cu2_t", bufs=1)
            nc.scalar.mul(cu2_t[:], c_t[:], float(2.0 ** -SZ2))
            for t in range(HM, FM):
                ch = (t - HM) % H2M
                cv = c_t if t < HM + H2M else (
                    cu1_t if t < HM + 2 * H2M else cu2_t)
                nc.vector.tensor_scalar_mul(
                    wTf_sb[:, t, :], wTf_sb[:, t, :], cv[:, ch:ch + 1])

            # outT[64, R] = wTf'.T @ jkT + (s0+bf) + s1 (x) rsA + s2 (x) rsA2
            def jk_rhs(t):
                if t < HM:
                    return hT_sb[:, t, :]
                if t < HM + H2M:
                    return zT_sb[:, t - HM, :]
                return u_sb[:, t - HM - H2M, :]

            outsb = tmp.tile([O, R], F32, name="outsb", bufs=1)
            for ci, (cs, cw) in enumerate(NCH):
                ops = ps.tile([O, 512], F32, name=f"ops_{ci}", tag=f"acc{4 + ci}")
                for t in range(FM):
                    nc.tensor.matmul(
                        ops[:, :cw], lhsT=wTf_sb[:, t, :],
                        rhs=jk_rhs(t)[:, cs:cs + cw],
                        start=(t == 0), stop=(t == FM - 1))
                nc.vector.tensor_scalar_add(
                    outsb[:, cs:cs + cw], ops[:, :cw], s0b[:])
            rk1 = tmp.tile([O, R], F32, name="rk1", tag="rk", bufs=1)
            nc.vector.tensor_scalar_mul(rk1[:], rsA_sb[:], s_cols[:, 1:2])
            nc.vector.tensor_add(out=outsb[:], in0=outsb[:], in1=rk1[:])
            rk2 = tmp.tile([O, R], F32, name="rk2", tag="rk", bufs=1)
            nc.vector.tensor_scalar_mul(rk2[:], rsA2_sb[:], s_cols[:, 2:3])
            nc.vector.tensor_add(out=outsb[:], in0=outsb[:], in1=rk2[:])

            # transpose [O, R] -> node-major [R, O] and write out
            o_nm = tmp.tile([P, RT, O], F32, name="o_nm", bufs=1)
            for nt in range(RT):
                tps32 = ps.tile([P, O], F32, name=f"otp_{nt}",
                                tag=f"acc{nt % 8}")
                nc.tensor.transpose(
                    tps32[:], outsb[:, nt * P:(nt + 1) * P], id32[:O, :O])
                nc.any.tensor_copy(out=o_nm[:, nt, :], in_=tps32[:])
            nc.sync.dma_start(
                out.ap().rearrange("(nt p) o -> p nt o", p=P), o_nm[:])

    nc.compile()
    return nc


_PROGRAM_CACHE = {}


def _get_program(NT, R, scales):
    key = (NT, R, scales)
    if key not in _PROGRAM_CACHE:
        _PROGRAM_CACHE[key] = build_program(NT, R, scales)
    return _PROGRAM_CACHE[key]


def _p2(cap, v):
    """Largest power-of-2 exponent s with v * 2^s <= cap."""
    return int(np.floor(np.log2(cap / max(float(v), 1e-30))))


def _prep(inputs):
    """Host-side shared prep: exact h, z column means, s-scales, exponents."""
    x = np.asarray(inputs["x"], np.float32)
    we = np.asarray(inputs["w_embed"], np.float32)
    be = np.asarray(inputs["b_embed"], np.float32)
    adj = np.asarray(inputs["adj_t"], np.float32)
    adj2 = np.asarray(inputs["adj_t2"], np.float32)
    NT = x.shape[0]
    h = np.maximum(x @ we.T + be, 0)
    # exact per-feature means of z = [A@h; A2@h] via the colsum identity
    muz = np.concatenate([(adj.sum(0) / NT) @ h, (adj2.sum(0) / NT) @ h])
    d1 = (adj > 0).sum(1).astype(np.float32)
    d2 = (adj2 > 0).sum(1).astype(np.float32)
    s1 = np.where(d1 > 0, 1.0 / np.sqrt(np.maximum(d1, 1e-12)), 0.0).astype(np.float32)
    s2 = np.where(d2 > 0, 1.0 / np.sqrt(np.maximum(d2, 1e-12)), 0.0).astype(np.float32)
    # safe bound for |z - muz|
    hmax = float(np.abs(h).max())
    zb = float(max(adj.sum(1).max(), adj2.sum(1).max())) * hmax \
        + float(np.abs(muz).max()) + 1e-6
    SZ1 = _p2(192.0, s1.max() * zb)
    SZ2 = _p2(192.0, s2.max() * zb)
    return dict(h=h, muz=muz, s1=s1, s2=s2, SZ1=SZ1, SZ2=SZ2)


def compute_scales(inputs, prep=None):
    prep = prep if prep is not None else _prep(inputs)
    return (prep["SZ1"], prep["SZ2"])


def make_in_maps(inputs, NT, R, scales, prep=None):
    """Shard full inputs into per-core input maps (host-side, numpy)."""
    SZ1, SZ2 = scales
    prep = prep if prep is not None else _prep(inputs)
    muz, s1, s2 = prep["muz"], prep["s1"], prep["s2"]
    F8NP = mybir.dt.np(F8)
    x = np.asarray(inputs["x"], np.float32)
    adj = np.asarray(inputs["adj_t"], np.float32)
    adj2 = np.asarray(inputs["adj_t2"], np.float32)
    we = np.asarray(inputs["w_embed"], np.float32)
    be = np.asarray(inputs["b_embed"], np.float32)
    gam = np.asarray(inputs["bn_gamma"], np.float32)
    bet = np.asarray(inputs["bn_beta"], np.float32)
    wf = np.asarray(inputs["w_fin"], np.float32)
    bf = np.asarray(inputs["b_fin"], np.float32)

    H2M = H2 // P
    RT = R // P
    xTf_h = np.ascontiguousarray(x.T).astype(np.float16)
    wTe_h = np.ascontiguousarray(we.T).astype(np.float16)
    be_h = np.ascontiguousarray(be.reshape(H // P, P).T).astype(np.float32)
    bebc_h = np.ascontiguousarray(
        np.broadcast_to(be[None, :], (P, H))).astype(np.float16)
    wTf_h = np.ascontiguousarray(wf.T).astype(np.float16)
    bff_h = np.ascontiguousarray(bf[:, None]).astype(np.float32)
    gam_h = np.ascontiguousarray(gam.reshape(H2M, P).T).astype(np.float32)
    bet_h = np.ascontiguousarray(bet.reshape(H2M, P).T).astype(np.float32)
    muz_h = np.ascontiguousarray(muz.reshape(H2M, P).T).astype(np.float32)
    nmuz_h = np.ascontiguousarray(-muz.reshape(H2M, P).T).astype(np.float32)

    KT = NT // P
    s1f_h = np.ascontiguousarray(s1.reshape(KT, P).T).astype(np.float32)
    s2f_h = np.ascontiguousarray(s2.reshape(KT, P).T).astype(np.float32)
    in_maps = []
    for r in range(NCORES):
        rows = slice(r * R, (r + 1) * R)
        rsA_h = np.ascontiguousarray(
            np.broadcast_to(adj[rows].sum(1)[None, :], (O, R))).astype(np.float16)
        rsA2_h = np.ascontiguousarray(
            np.broadcast_to(adj2[rows].sum(1)[None, :], (O, R))).astype(np.float16)
        s1r, s2r = s1[rows], s2[rows]
        in_maps.append({
            "xTf": xTf_h,
            "xT": np.ascontiguousarray(x[rows].T).astype(np.float16),
            "abT": np.ascontiguousarray(
                (adj[rows] > 0).T.astype(np.float32)).astype(F8NP),
            "abT2": np.ascontiguousarray(
                (adj2[rows] > 0).T.astype(np.float32)).astype(F8NP),
            "wTe": wTe_h, "be": be_h, "bebc": bebc_h, "wTf": wTf_h,
            "bff": bff_h, "gam": gam_h, "bet": bet_h,
            "nmuz": nmuz_h, "muz": muz_h,
            "s1q": np.ascontiguousarray(
                (s1r * 2.0 ** SZ1).reshape(RT, P).T).astype(np.float32),
            "s2q": np.ascontiguousarray(
                (s2r * 2.0 ** SZ2).reshape(RT, P).T).astype(np.float32),
            "s1f": s1f_h, "s2f": s2f_h,
            "s1c": np.ascontiguousarray(np.broadcast_to(
                s1r[None, :], (P, R))).astype(np.float16),
            "s2c": np.ascontiguousarray(np.broadcast_to(
                s2r[None, :], (P, R))).astype(np.float16),
            "rsA": rsA_h, "rsA2": rsA2_h,
        })
    return in_maps


def kernel(**inputs):
    NT, R = FULL_CFG["NT"], FULL_CFG["R"]
    prep = _prep(inputs)
    scales = compute_scales(inputs, prep)
    nc = _get_program(NT, R, scales)
    in_maps = make_in_maps(inputs, NT, R, scales, prep)
    res = run_bass_kernel_spmd(nc, in_maps, core_ids=list(range(NCORES)))
    out = np.concatenate(
        [res.results[r]["out"] for r in range(NCORES)], axis=0)
    return out.astype(np.float32)


# revision 18
# speedup vs baseline: 1.1317x; 1.1317x over previous
"""H2GCN forward pass on 8 Trainium2 NeuronCores (Bass/Tile SPMD kernel).

Strategy (1D row-parallel SpMM; fp16 conv1, fp8-DoubleRow conv2):
  - Nodes are sharded across 8 cores (1024 rows each). Each core receives
    column-slices adjT = adj[rows, :].T ([8192, 1024]) of both normalized
    adjacency matrices in fp16 (conv1) and of the BINARY patterns in fp8
    (conv2). The gcn normalization A = S*Abin*S (S = diag(1/sqrt(deg)))
    is factored out for conv2: Abin is EXACT in fp8 (entries 0/1), the
    per-node s-scales fold into the z quantization (pre-scale) and the
    PSUM->SBUF copies (post-scale), so conv2's adjacency contributes zero
    quantization error while running at fp8 DoubleRow speed (2 fp8 weights
    per PE cell, 2 contraction k-tiles per matmul, ~1.8x fp16 rate).
  - The feature embed is *replicated* (fp16): every core computes the full h
    for all 8192 nodes in node-major layout instead of all-gathering it.
  - conv1 (fp16, full precision path): zT = [A@h; A2@h].T feature-major, RAW.
  - BatchNorm is *algebraically absorbed* into the final projection:
        z_n = z*c + d  with c = gamma*rsqrt(var+eps), d = beta - mean*c
        A@z_n = (A@z)*c + rowsum(A) (x) d
    so conv2 runs on raw z and the final projection applies per-row scaled
    weights plus rank-1 corrections from host-exact rowsums.
  - z is MEAN-CENTERED (host-exact colmeans mu_z via the colsum identity
    mean(A@h) = (colsum(A)/N)@h) before fp8 quantization: the quantization
    error then scales with the per-column FLUCTUATION - exactly what BN
    normalizes by - instead of the column mean, avoiding the ~30x error
    amplification of the near-constant A2@h columns. The centering is
    algebraically exact: A@z = A@(z-mu) + rowsum(A) (x) mu, and the rank-1
    correction folds into the existing BN-absorption vector (d' = c*mu + d)
    at ZERO device cost.
  - Centered z is transposed to node-major, quantized twice (s1- and
    s2-pre-scaled fp8), and AllGathered in two feature-halves into
    addr_space="Shared" DRAM (fast HBM-HBM collective path), with
    partition-swizzled layout so gather-in and SBUF reload DMAs are
    contiguous 2KB-per-partition runs. The reloads run on the sync queue so
    they are NOT serialized behind the BN AllReduce on the gpsimd/CC queue.
    z1's gather hides under conv1's second half, z2's under conv2's first
    m-pass. BN statistics use a tiny AllReduce, off the critical path.
"""

import numpy as np

import concourse.bass as bass
import concourse.mybir as mybir
import concourse.tile as tile
from concourse import bacc
from concourse.bass_utils import run_bass_kernel_spmd
from concourse.masks import make_identity

P = 128
NCORES = 8
BN_EPS = 1e-5

F8 = mybir.dt.float8e4
F16 = mybir.dt.float16
F32 = mybir.dt.float32
DR = mybir.MatmulPerfMode.DoubleRow

FULL_CFG = dict(NT=8192, R=1024)
IN_CH = 512   # input features
H = 256       # hidden
H2 = 512      # 2*H (BN width)
O = 64        # output features
F = 7 * H     # 1792, JK concat width


def _nchunks(R):
    """Split the per-core node free-dim R into <=512 chunks (PSUM bank width)."""
    out = []
    s = 0
    while s < R:
        w = min(512, R - s)
        out.append((s, w))
        s += w
    return out


def build_program(NT, R, scales):
    """Build the SPMD Bass program. NT = total nodes, R = rows per core.
    scales = (SZ1, SZ2): power-of-2 exponents for the fp8 z quantization."""
    SZ1, SZ2 = scales
    KT = NT // P           # node k-tiles (contraction tiles)
    K2 = KT // 2           # DoubleRow k-tile pairs
    RT = R // P            # per-core node tiles (free-dim tiles / transposes)
    NCH = _nchunks(R)
    NC2 = len(NCH)
    HM = H // P            # 2  (hidden chunks)
    H2M = H2 // P          # 4
    FM = F // P            # 14
    INK = IN_CH // P       # 4

    nc = bacc.Bacc("TRN2", target_bir_lowering=False, debug=False,
                   num_devices=NCORES)

    # --- I/O -------------------------------------------------------------
    xTf = nc.dram_tensor("xTf", [IN_CH, NT], F16, kind="ExternalInput")
    xT = nc.dram_tensor("xT", [IN_CH, R], F16, kind="ExternalInput")
    abT = nc.dram_tensor("abT", [NT, R], F8, kind="ExternalInput")
    abT2 = nc.dram_tensor("abT2", [NT, R], F8, kind="ExternalInput")
    wTe = nc.dram_tensor("wTe", [IN_CH, H], F16, kind="ExternalInput")
    be = nc.dram_tensor("be", [P, HM], F32, kind="ExternalInput")
    bebc = nc.dram_tensor("bebc", [P, H], F16, kind="ExternalInput")
    wTf = nc.dram_tensor("wTf", [F, O], F16, kind="ExternalInput")
    bff = nc.dram_tensor("bff", [O, 1], F32, kind="ExternalInput")
    gam = nc.dram_tensor("gam", [P, H2M], F32, kind="ExternalInput")
    bet = nc.dram_tensor("bet", [P, H2M], F32, kind="ExternalInput")
    nmuz = nc.dram_tensor("nmuz", [P, H2M], F32, kind="ExternalInput")
    muz = nc.dram_tensor("muz", [P, H2M], F32, kind="ExternalInput")
    s1q = nc.dram_tensor("s1q", [P, RT], F32, kind="ExternalInput")
    s2q = nc.dram_tensor("s2q", [P, RT], F32, kind="ExternalInput")
    s1f = nc.dram_tensor("s1f", [P, KT], F32, kind="ExternalInput")
    s2f = nc.dram_tensor("s2f", [P, KT], F32, kind="ExternalInput")
    s1c = nc.dram_tensor("s1c", [P, R], F16, kind="ExternalInput")
    s2c = nc.dram_tensor("s2c", [P, R], F16, kind="ExternalInput")
    rsA = nc.dram_tensor("rsA", [O, R], F16, kind="ExternalInput")
    rsA2 = nc.dram_tensor("rsA2", [O, R], F16, kind="ExternalInput")
    out = nc.dram_tensor("out", [R, O], F32, kind="ExternalOutput")

    rg = [list(range(NCORES))]

    with tile.TileContext(nc) as tc:
        with (
            tc.tile_pool(name="const", bufs=1) as const,
            tc.tile_pool(name="feat", bufs=1) as feat,
            tc.tile_pool(name="tmp", bufs=2) as tmp,
            tc.tile_pool(name="stream", bufs=10) as stream,
            tc.tile_pool(name="ps", bufs=1, space="PSUM") as ps,
            tc.tile_pool(name="dram", bufs=1, space="DRAM") as dram,
        ):
            # --- CC warm-up: a tiny AllGather absorbs the collective
            # init barrier (~45us) long before the z gathers need the CC --
            wu_sb = const.tile([P, 1], F32, name="wu_sb")
            nc.vector.memset(wu_sb[:], 1.0)
            wu_in = dram.tile([P, 1], F32, name="wu_in")
            nc.gpsimd.dma_start(wu_in[:], wu_sb[:])
            wu_out = dram.tile([NCORES, P, 1], F32, name="wu_out",
                               addr_space="Shared")
            nc.gpsimd.collective_compute(
                "AllGather", mybir.AluOpType.bypass, replica_groups=rg,
                ins=[wu_in.opt()], outs=[wu_out.opt()],
            )

            # --- constants / weights (embed-critical ones first) --------
            wTe_sb = const.tile([P, INK, H], F16, name="wTe_sb")
            nc.sync.dma_start(wTe_sb[:], wTe.ap().rearrange("(k p) m -> p k m", p=P))
            bebc_sb = const.tile([P, H], F16, name="bebc_sb")
            nc.sync.dma_start(bebc_sb[:], bebc.ap())
            s1f_sb = const.tile([P, KT], F32, name="s1f_sb")
            nc.sync.dma_start(s1f_sb[:], s1f.ap())
            s2f_sb = const.tile([P, KT], F32, name="s2f_sb")
            nc.sync.dma_start(s2f_sb[:], s2f.ap())

            # full x.T, staged through a double-buffer (32 KB instead of
            # 64 KB resident) so the embed starts early and SBUF stays small
            xTf_t = xTf.ap().rearrange("(k p) n -> p k n", p=P)
            XG = 2048
            KG = XG // P

            # --- phase B1: replicated full embed, node-major ------------
            # hs_a[node, feat] = s_a[node] * relu(x @ w_embed.T + b), the
            # row-scaled h needed by the factored conv1 (A = S Abin S).
            # relu(s*x) = s*relu(x) for s>=0, so the s-scale rides the relu.
            hs1_sb = feat.tile([P, KT, H], F16, name="hs1_sb", tag="kxnA")
            hs2_sb = feat.tile([P, KT, H], F16, name="hs2_sb", tag="hs2")
            xg_sb = None
            for k in range(KT):
                if k % KG == 0:
                    g = k * P
                    xg_sb = feat.tile([P, INK, XG], F16, name=f"xg_{k}",
                                      tag="xg", bufs=2)
                    nc.sync.dma_start(xg_sb[:], xTf_t[:, :, g:g + XG])
                hps = ps.tile([P, H], F32, name=f"hps_{k}", tag=f"acc{k % 8}")
                for t in range(INK):
                    nc.tensor.matmul(
                        hps[:],
                        lhsT=xg_sb[:, t, (k % KG) * P:(k % KG + 1) * P],
                        rhs=wTe_sb[:, t, :],
                        start=(t == 0), stop=(t == INK - 1),
                    )
                nc.vector.tensor_tensor(
                    out=hps[:], in0=hps[:], in1=bebc_sb[:],
                    op=mybir.AluOpType.add)
                nc.scalar.activation(
                    hs1_sb[:, k, :], hps[:],
                    mybir.ActivationFunctionType.Relu,
                    scale=s1f_sb[:, k:k + 1])
                nc.vector.tensor_scalar(
                    out=hs2_sb[:, k, :], in0=hps[:],
                    scalar1=0.0, scalar2=s2f_sb[:, k:k + 1],
                    op0=mybir.AluOpType.max, op1=mybir.AluOpType.mult)

            # --- remaining constants (needed from B2 / conv1 onward) ----
            xT_sb = const.tile([P, INK, R], F16, name="xT_sb")
            nc.sync.dma_start(xT_sb[:], xT.ap().rearrange("(k p) n -> p k n", p=P))
            be_sb = const.tile([P, HM], F32, name="be_sb")
            nc.sync.dma_start(be_sb[:], be.ap())
            id16 = const.tile([P, P], F16, name="id16")
            make_identity(nc, id16)
            id32 = const.tile([P, P], F32, name="id32")
            make_identity(nc, id32)
            nmuz_sb = const.tile([P, H2M], F32, name="nmuz_sb")
            nc.sync.dma_start(nmuz_sb[:], nmuz.ap())
            s1q_sb = const.tile([P, RT], F32, name="s1q_sb")
            nc.sync.dma_start(s1q_sb[:], s1q.ap())
            s2q_sb = const.tile([P, RT], F32, name="s2q_sb")
            nc.sync.dma_start(s2q_sb[:], s2q.ap())
            s1c_sb = const.tile([P, R], F16, name="s1c_sb")
            nc.sync.dma_start(s1c_sb[:], s1c.ap())
            s2c_sb = const.tile([P, R], F16, name="s2c_sb")
            nc.sync.dma_start(s2c_sb[:], s2c.ap())

            # --- phase B2: local embed, feature-major (for the JK concat)
            hT_sb = feat.tile([P, HM, R], F16, name="hT_sb")
            for m in range(HM):
                for ci, (cs, cw) in enumerate(NCH):
                    eps_t = ps.tile([P, 512], F32, name=f"eps_{m}_{ci}",
                                    tag=f"acc{(m * NC2 + ci) % 8}")
                    for t in range(INK):
                        nc.tensor.matmul(
                            eps_t[:, :cw],
                            lhsT=wTe_sb[:, t, m * P:(m + 1) * P],
                            rhs=xT_sb[:, t, cs:cs + cw],
                            start=(t == 0), stop=(t == INK - 1),
                        )
                    nc.scalar.activation(
                        hT_sb[:, m, cs:cs + cw], eps_t[:, :cw],
                        mybir.ActivationFunctionType.Relu,
                        bias=be_sb[:, m:m + 1],
                    )

            # --- phase D: conv1 (factored: fp16 hs x fp8 binary Abin) ---
            # z = S (Abin @ (S h)): the streamed rhs is the EXACT binary
            # pattern in fp8 (half the bytes of fp16 normalized A, zero
            # quantization error); the output s-post-scale folds into the
            # PSUM->SBUF copy. One adjacency half at a time; each half's
            # centered transposes + dual fp8 quantize + AllGather overlap
            # the next compute.
            zT_sb = feat.tile([P, H2M, R], F16, name="zT_sb")
            zag_out = [None, None]
            for half, (src, hs_sb, sc_sb) in (
                    (0, (abT, None, None)), (1, (abT2, None, None))):
                hs_sb = hs1_sb if half == 0 else hs2_sb
                sc_sb = s1c_sb if half == 0 else s2c_sb
                zps = {}
                for m in range(HM):
                    for ci in range(NC2):
                        zps[(m, ci)] = ps.tile(
                            [P, 512], F32, name=f"zps_{half}_{m}_{ci}",
                            tag=f"acc{(half * 4 + m * NC2 + ci) % 8}")
                for k in range(KT):
                    at = stream.tile([P, R], F8, name=f"c1_{half}_{k}", tag="adj")
                    nc.sync.dma_start(at[:], src[k * P:(k + 1) * P, :])
                    for m in range(HM):
                        for ci, (cs, cw) in enumerate(NCH):
                            nc.tensor.matmul(
                                zps[(m, ci)][:, :cw],
                                lhsT=hs_sb[:, k, m * P:(m + 1) * P],
                                rhs=at[:, cs:cs + cw],
                                start=(k == 0), stop=(k == KT - 1),
                            )
                # raw z = s_out * PSUM (vector) + centered z (scalar)
                zcT = tmp.tile([P, HM, R], F16, name="zcT", tag="zcT", bufs=1)
                for m in range(HM):
                    for ci, (cs, cw) in enumerate(NCH):
                        nc.vector.tensor_tensor(
                            out=zT_sb[:, half * HM + m, cs:cs + cw],
                            in0=zps[(m, ci)][:, :cw],
                            in1=sc_sb[:, cs:cs + cw],
                            op=mybir.AluOpType.mult)
                        nc.scalar.activation(
                            zcT[:, m, cs:cs + cw],
                            zT_sb[:, half * HM + m, cs:cs + cw],
                            mybir.ActivationFunctionType.Identity,
                            bias=nmuz_sb[:, half * HM + m:half * HM + m + 1])

                # transpose centered z to node-major and quantize twice
                # (s1- and s2-pre-scaled fp8), then AllGather both
                zs_nm = tmp.tile([P, RT, 2, H], F8, name="znm", tag="znm", bufs=1)
                for fi in range(HM):
                    for nt in range(RT):
                        tps = ps.tile(
                            [P, P], F16, name=f"ztp_{half}_{fi}_{nt}",
                            tag=f"acc{(fi * RT + nt) % 4 + half * 4}")
                        nc.tensor.transpose(
                            tps[:], zcT[:, fi, nt * P:(nt + 1) * P],
                            id16[:])
                        nc.scalar.mul(
                            zs_nm[:, nt, 0, fi * P:(fi + 1) * P], tps[:],
                            s1q_sb[:, nt:nt + 1])
                        nc.scalar.mul(
                            zs_nm[:, nt, 1, fi * P:(fi + 1) * P], tps[:],
                            s2q_sb[:, nt:nt + 1])
                zin = dram.tile([P, RT, 2, H], F8, name=f"zag_in_{half}")
                nc.gpsimd.dma_start(zin[:], zs_nm[:])
                zout = dram.tile([NCORES, P, RT, 2, H], F8,
                                 name=f"zag_out_{half}", addr_space="Shared")
                nc.gpsimd.collective_compute(
                    "AllGather", mybir.AluOpType.bypass, replica_groups=rg,
                    ins=[zin.opt()], outs=[zout.opt()],
                )
                zag_out[half] = zout

            # --- gathered-z reloads: scalar queue (half 0, idle there
            # after conv1) and sync queue (half 1); emitted BEFORE phase E
            # so no BN-coefficient wait can block them. Layout [p][kt][a][f]
            # with kt=(r,nt): fully contiguous 4KB runs per (r,p).
            zfd_sb = []
            for half, eng in ((0, nc.scalar), (1, nc.sync)):
                zfd = feat.tile([P, KT, 2, H], F8, name=f"zfd_{half}",
                                tag=("hs2" if half == 0 else "kxnA"))
                eng.dma_start(
                    zfd[:].rearrange("p (r nt) a f -> p r nt a f", nt=RT),
                    zag_out[half].rearrange("r p nt a f -> p r nt a f"))
                zfd_sb.append(zfd)

            # --- phase E: BN stats + AllReduce (off critical path) ------
            stat_sb = tmp.tile([P, 2 * H2M], F32, name="stat_sb", bufs=1)
            for f in range(H2M):
                sq = tmp.tile([P, R], F16, name="sq", tag="zcT", bufs=1)
                nc.scalar.activation(
                    sq[:], zT_sb[:, f, :], mybir.ActivationFunctionType.Copy,
                    accum_out=stat_sb[:, f:f + 1])
                sq2 = tmp.tile([P, R], F16, name="sq2", tag="zcT", bufs=1)
                nc.scalar.activation(
                    sq2[:], zT_sb[:, f, :], mybir.ActivationFunctionType.Square,
                    accum_out=stat_sb[:, H2M + f:H2M + f + 1])
            ar_in = dram.tile([P, 2 * H2M], F32, name="ar_in")
            nc.gpsimd.dma_start(ar_in[:], stat_sb[:])
            ar_out = dram.tile([P, 2 * H2M], F32, name="ar_out")
            nc.gpsimd.collective_compute(
                "AllReduce", mybir.AluOpType.add, replica_groups=rg,
                ins=[ar_in.opt()], outs=[ar_out.opt()],
            )
            stat_g = tmp.tile([P, 2 * H2M], F32, name="stat_g", bufs=1)
            nc.gpsimd.dma_start(stat_g[:], ar_out[:])

            # BN coefficients c, d (feature-major [128, 4], fp32)
            gam_sb = const.tile([P, H2M], F32, name="gam_sb")
            nc.sync.dma_start(gam_sb[:], gam.ap())
            bet_sb = const.tile([P, H2M], F32, name="bet_sb")
            nc.sync.dma_start(bet_sb[:], bet.ap())
            muz_sb = const.tile([P, H2M], F32, name="muz_sb")
            nc.sync.dma_start(muz_sb[:], muz.ap())
            cmean = tmp.tile([P, H2M], F32, name="cmean", bufs=1)
            nc.scalar.mul(cmean[:], stat_g[:, 0:H2M], 1.0 / NT)
            cvar = tmp.tile([P, H2M], F32, name="cvar", bufs=1)
            nc.scalar.mul(cvar[:], stat_g[:, H2M:2 * H2M], 1.0 / NT)
            msq = tmp.tile([P, H2M], F32, name="msq", bufs=1)
            nc.vector.tensor_mul(out=msq[:], in0=cmean[:], in1=cmean[:])
            nc.vector.tensor_tensor(
                out=cvar[:], in0=cvar[:], in1=msq[:],
                op=mybir.AluOpType.subtract)
            eps_sb = tmp.tile([P, 1], F32, name="eps_sb", bufs=1)
            nc.vector.memset(eps_sb[:], BN_EPS)
            cstd = tmp.tile([P, H2M], F32, name="cstd", bufs=1)
            nc.scalar.activation(
                cstd[:], cvar[:], mybir.ActivationFunctionType.Sqrt,
                bias=eps_sb[:])
            crstd = tmp.tile([P, H2M], F32, name="crstd", bufs=1)
            nc.vector.reciprocal(crstd[:], cstd[:])
            c_t = tmp.tile([P, H2M], F32, name="c_t", bufs=1)
            nc.vector.tensor_mul(out=c_t[:], in0=crstd[:], in1=gam_sb[:])
            d_t = tmp.tile([P, H2M], F32, name="d_t", bufs=1)
            nc.vector.tensor_mul(out=d_t[:], in0=cmean[:], in1=c_t[:])
            nc.vector.tensor_tensor(
                out=d_t[:], in0=bet_sb[:], in1=d_t[:],
                op=mybir.AluOpType.subtract)
            d16 = tmp.tile([P, H2M], F16, name="d16", bufs=1)
            nc.vector.tensor_copy(out=d16[:], in_=d_t[:])
            # d' = c * mu_z + d  (U-block correction incl. the z centering)
            dp_t = tmp.tile([P, H2M], F32, name="dp_t", bufs=1)
            nc.vector.tensor_mul(out=dp_t[:], in0=c_t[:], in1=muz_sb[:])
            nc.vector.tensor_add(out=dp_t[:], in0=dp_t[:], in1=d_t[:])
            d16p = tmp.tile([P, H2M], F16, name="d16p", bufs=1)
            nc.vector.tensor_copy(out=d16p[:], in_=dp_t[:])

            # --- phase F: conv2 on centered z (fp8 DoubleRow), two ------
            # m-half passes; pass 0 (z features 0:256, from z1) only needs
            # zag_out[0], so it overlaps z2's AllGather. Each pass streams
            # both binary adjacencies.
            ab_p = abT.ap().rearrange("(k2 ko p) n -> p k2 ko n", ko=2, p=P)
            ab2_p = abT2.ap().rearrange("(k2 ko p) n -> p k2 ko n", ko=2, p=P)
            u_sb = feat.tile([P, 2 * H2M, R], F16, name="u_sb")
            for half in (0, 1):
                zfd = zfd_sb[half]
                ups = {}
                for a in (0, 1):
                    for m in range(HM):
                        for ci in range(NC2):
                            ups[(a, m, ci)] = ps.tile(
                                [P, 512], F32, name=f"ups_{half}_{a}_{m}_{ci}",
                                tag=f"acc{(a * 4 + m * NC2 + ci) % 8}")
                for k2 in range(K2):
                    at = stream.tile([P, 2, R], F8, name=f"c2a_{half}_{k2}",
                                     tag="adj")
                    nc.sync.dma_start(at[:], ab_p[:, k2])
                    at2 = stream.tile([P, 2, R], F8, name=f"c2b_{half}_{k2}",
                                      tag="adj")
                    nc.sync.dma_start(at2[:], ab2_p[:, k2])
                    for m in range(HM):
                        for ci, (cs, cw) in enumerate(NCH):
                            nc.tensor.matmul(
                                ups[(0, m, ci)][:, :cw],
                                lhsT=zfd[:, 2 * k2:2 * k2 + 2, 0,
                                         m * P:(m + 1) * P],
                                rhs=at[:, :, cs:cs + cw],
                                start=(k2 == 0), stop=(k2 == K2 - 1),
                                perf_mode=DR,
                            )
                            nc.tensor.matmul(
                                ups[(1, m, ci)][:, :cw],
                                lhsT=zfd[:, 2 * k2:2 * k2 + 2, 1,
                                         m * P:(m + 1) * P],
                                rhs=at2[:, :, cs:cs + cw],
                                start=(k2 == 0), stop=(k2 == K2 - 1),
                                perf_mode=DR,
                            )
                # u feature layout: chunks 0..3 = U1 (A@zc), 4..7 = U2
                # (A2@zc); this pass produces z-feature chunks {half*2,
                # half*2+1} of each. Post-scale s_a * 2^-SZa (DVE).
                for a, su in ((0, s1c_sb), (1, s2c_sb)):
                    for m in range(HM):
                        for ci, (cs, cw) in enumerate(NCH):
                            nc.vector.tensor_tensor(
                                out=u_sb[:, a * H2M + half * HM + m, cs:cs + cw],
                                in0=ups[(a, m, ci)][:, :cw],
                                in1=su[:, cs:cs + cw],
                                op=mybir.AluOpType.mult)

            # --- phase G: final projection with absorbed BN -------------
            wTf_sb = const.tile([P, FM, O], F16, name="wTf_sb")
            nc.sync.dma_start(wTf_sb[:], wTf.ap().rearrange("(k p) m -> p k m", p=P))
            bff_sb = const.tile([O, 1], F32, name="bff_sb")
            nc.sync.dma_start(bff_sb[:], bff.ap())
            rsA_sb = const.tile([O, R], F16, name="rsA_sb")
            nc.sync.dma_start(rsA_sb[:], rsA.ap())
            rsA2_sb = const.tile([O, R], F16, name="rsA2_sb")
            nc.sync.dma_start(rsA2_sb[:], rsA2.ap())

            # s_0 = W_zn @ d ; s_j = W_Uj @ d' (j=1,2), from UNSCALED wTf
            s_cols = tmp.tile([O, 3], F32, name="s_cols", bufs=1)
            for j, (base, dv) in enumerate((
                    (HM, d16), (HM + H2M, d16p), (HM + 2 * H2M, d16p))):
                sps = ps.tile([O, 1], F32, name=f"sps_{j}", tag=f"acc{j}")
                for t in range(H2M):
                    nc.tensor.matmul(
                        sps[:], lhsT=wTf_sb[:, base + t, :],
                        rhs=dv[:, t:t + 1],
                        start=(t == 0), stop=(t == H2M - 1))
                nc.vector.tensor_copy(out=s_cols[:, j:j + 1], in_=sps[:])
            s0b = tmp.tile([O, 1], F32, name="s0b", bufs=1)
            nc.vector.tensor_add(out=s0b[:], in0=s_cols[:, 0:1], in1=bff_sb[:])

            # scale wTf rows in place: z_n block by c, U blocks by
            # c * 2^-SZa (u_sb is stored 2^SZa-scaled; descale folds here)
            cu1_t = tmp.tile([P, H2M], F32, name="cu1_t", bufs=1)
            nc.scalar.mul(cu1_t[:], c_t[:], float(2.0 ** -SZ1))
            cu2_t = tmp.tile([P, H2M], F32, name="cu2_t", bufs=1)
            nc.scalar.mul(cu2_t[:], c_t[:], float(2.0 ** -SZ2))
            for t in range(HM, FM):
                ch = (t - HM) % H2M
                cv = c_t if t < HM + H2M else (
                    cu1_t if t < HM + 2 * H2M else cu2_t)
                nc.vector.tensor_scalar_mul(
                    wTf_sb[:, t, :], wTf_sb[:, t, :], cv[:, ch:ch + 1])

            # outT[64, R] = wTf'.T @ jkT + (s0+bf) + s1 (x) rsA + s2 (x) rsA2
            def jk_rhs(t):
                if t < HM:
                    return hT_sb[:, t, :]
                if t < HM + H2M:
                    return zT_sb[:, t - HM, :]
                return u_sb[:, t - HM - H2M, :]

            outsb = tmp.tile([O, R], F32, name="outsb", bufs=1)
            for ci, (cs, cw) in enumerate(NCH):
                ops = ps.tile([O, 512], F32, name=f"ops_{ci}", tag=f"acc{4 + ci}")
                for t in range(FM):
                    nc.tensor.matmul(
                        ops[:, :cw], lhsT=wTf_sb[:, t, :],
                        rhs=jk_rhs(t)[:, cs:cs + cw],
                        start=(t == 0), stop=(t == FM - 1))
                nc.vector.tensor_scalar_add(
                    outsb[:, cs:cs + cw], ops[:, :cw], s0b[:])
            rk1 = tmp.tile([O, R], F32, name="rk1", tag="rk", bufs=1)
            nc.vector.tensor_scalar_mul(rk1[:], rsA_sb[:], s_cols[:, 1:2])
            nc.vector.tensor_add(out=outsb[:], in0=outsb[:], in1=rk1[:])
            rk2 = tmp.tile([O, R], F32, name="rk2", tag="rk", bufs=1)
            nc.vector.tensor_scalar_mul(rk2[:], rsA2_sb[:], s_cols[:, 2:3])
            nc.vector.tensor_add(out=outsb[:], in0=outsb[:], in1=rk2[:])

            # transpose [O, R] -> node-major [R, O] and write out
            o_nm = tmp.tile([P, RT, O], F32, name="o_nm", bufs=1)
            for nt in range(RT):
                tps32 = ps.tile([P, O], F32, name=f"otp_{nt}",
                                tag=f"acc{nt % 8}")
                nc.tensor.transpose(
                    tps32[:], outsb[:, nt * P:(nt + 1) * P], id32[:O, :O])
                nc.any.tensor_copy(out=o_nm[:, nt, :], in_=tps32[:])
            nc.sync.dma_start(
                out.ap().rearrange("(nt p) o -> p nt o", p=P), o_nm[:])

    nc.compile()
    return nc


_PROGRAM_CACHE = {}


def _get_program(NT, R, scales):
    key = (NT, R, scales)
    if key not in _PROGRAM_CACHE:
        _PROGRAM_CACHE[key] = build_program(NT, R, scales)
    return _PROGRAM_CACHE[key]


def _p2(cap, v):
    """Largest power-of-2 exponent s with v * 2^s <= cap."""
    return int(np.floor(np.log2(cap / max(float(v), 1e-30))))


def _prep(inputs):
    """Host-side shared prep: exact h, z column means, s-scales, exponents."""
    x = np.asarray(inputs["x"], np.float32)
    we = np.asarray(inputs["w_embed"], np.float32)
    be = np.asarray(inputs["b_embed"], np.float32)
    adj = np.asarray(inputs["adj_t"], np.float32)
    adj2 = np.asarray(inputs["adj_t2"], np.float32)
    NT = x.shape[0]
    h = np.maximum(x @ we.T + be, 0)
    # exact per-feature means of z = [A@h; A2@h] via the colsum identity
    muz = np.concatenate([(adj.sum(0) / NT) @ h, (adj2.sum(0) / NT) @ h])
    d1 = (adj > 0).sum(1).astype(np.float32)
    d2 = (adj2 > 0).sum(1).astype(np.float32)
    s1 = np.where(d1 > 0, 1.0 / np.sqrt(np.maximum(d1, 1e-12)), 0.0).astype(np.float32)
    s2 = np.where(d2 > 0, 1.0 / np.sqrt(np.maximum(d2, 1e-12)), 0.0).astype(np.float32)
    # safe bound for |z - muz|
    hmax = float(np.abs(h).max())
    zb = float(max(adj.sum(1).max(), adj2.sum(1).max())) * hmax \
        + float(np.abs(muz).max()) + 1e-6
    SZ1 = _p2(192.0, s1.max() * zb)
    SZ2 = _p2(192.0, s2.max() * zb)
    return dict(h=h, muz=muz, s1=s1, s2=s2, SZ1=SZ1, SZ2=SZ2)


def compute_scales(inputs, prep=None):
    prep = prep if prep is not None else _prep(inputs)
    return (prep["SZ1"], prep["SZ2"])


def make_in_maps(inputs, NT, R, scales, prep=None):
    """Shard full inputs into per-core input maps (host-side, numpy)."""
    SZ1, SZ2 = scales
    prep = prep if prep is not None else _prep(inputs)
    muz, s1, s2 = prep["muz"], prep["s1"], prep["s2"]
    F8NP = mybir.dt.np(F8)
    x = np.asarray(inputs["x"], np.float32)
    adj = np.asarray(inputs["adj_t"], np.float32)
    adj2 = np.asarray(inputs["adj_t2"], np.float32)
    we = np.asarray(inputs["w_embed"], np.float32)
    be = np.asarray(inputs["b_embed"], np.float32)
    gam = np.asarray(inputs["bn_gamma"], np.float32)
    bet = np.asarray(inputs["bn_beta"], np.float32)
    wf = np.asarray(inputs["w_fin"], np.float32)
    bf = np.asarray(inputs["b_fin"], np.float32)

    H2M = H2 // P
    RT = R // P
    xTf_h = np.ascontiguousarray(x.T).astype(np.float16)
    wTe_h = np.ascontiguousarray(we.T).astype(np.float16)
    be_h = np.ascontiguousarray(be.reshape(H // P, P).T).astype(np.float32)
    bebc_h = np.ascontiguousarray(
        np.broadcast_to(be[None, :], (P, H))).astype(np.float16)
    wTf_h = np.ascontiguousarray(wf.T).astype(np.float16)
    bff_h = np.ascontiguousarray(bf[:, None]).astype(np.float32)
    gam_h = np.ascontiguousarray(gam.reshape(H2M, P).T).astype(np.float32)
    bet_h = np.ascontiguousarray(bet.reshape(H2M, P).T).astype(np.float32)
    muz_h = np.ascontiguousarray(muz.reshape(H2M, P).T).astype(np.float32)
    nmuz_h = np.ascontiguousarray(-muz.reshape(H2M, P).T).astype(np.float32)

    KT = NT // P
    s1f_h = np.ascontiguousarray(s1.reshape(KT, P).T).astype(np.float32)
    s2f_h = np.ascontiguousarray(s2.reshape(KT, P).T).astype(np.float32)
    in_maps = []
    for r in range(NCORES):
        rows = slice(r * R, (r + 1) * R)
        rsA_h = np.ascontiguousarray(
            np.broadcast_to(adj[rows].sum(1)[None, :], (O, R))).astype(np.float16)
        rsA2_h = np.ascontiguousarray(
            np.broadcast_to(adj2[rows].sum(1)[None, :], (O, R))).astype(np.float16)
        s1r, s2r = s1[rows], s2[rows]
        in_maps.append({
            "xTf": xTf_h,
            "xT": np.ascontiguousarray(x[rows].T).astype(np.float16),
            "abT": np.ascontiguousarray(
                (adj[rows] > 0).T.astype(np.float32)).astype(F8NP),
            "abT2": np.ascontiguousarray(
                (adj2[rows] > 0).T.astype(np.float32)).astype(F8NP),
            "wTe": wTe_h, "be": be_h, "bebc": bebc_h, "wTf": wTf_h,
            "bff": bff_h, "gam": gam_h, "bet": bet_h,
            "nmuz": nmuz_h, "muz": muz_h,
            "s1q": np.ascontiguousarray(
                (s1r * 2.0 ** SZ1).reshape(RT, P).T).astype(np.float32),
            "s2q": np.ascontiguousarray(
                (s2r * 2.0 ** SZ2).reshape(RT, P).T).astype(np.float32),
            "s1f": s1f_h, "s2f": s2f_h,
            "s1c": np.ascontiguousarray(np.broadcast_to(
                s1r[None, :], (P, R))).astype(np.float16),
            "s2c": np.ascontiguousarray(np.broadcast_to(
                s2r[None, :], (P, R))).astype(np.float16),
            "rsA": rsA_h, "rsA2": rsA2_h,
        })
    return in_maps


def kernel(**inputs):
    NT, R = FULL_CFG["NT"], FULL_CFG["R"]
    prep = _prep(inputs)
    scales = compute_scales(inputs, prep)
    nc = _get_program(NT, R, scales)
    in_maps = make_in_maps(inputs, NT, R, scales, prep)
    res = run_bass_kernel_spmd(nc, in_maps, core_ids=list(range(NCORES)))
    out = np.concatenate(
        [res.results[r]["out"] for r in range(NCORES)], axis=0)
    return out.astype(np.float32)


# revision 20
# speedup vs baseline: 1.1598x; 1.0249x over previous
"""H2GCN forward pass on 8 Trainium2 NeuronCores (Bass/Tile SPMD kernel).

Strategy (1D row-parallel SpMM; fp16 conv1, fp8-DoubleRow conv2):
  - Nodes are sharded across 8 cores (1024 rows each). Each core receives
    column-slices adjT = adj[rows, :].T ([8192, 1024]) of both normalized
    adjacency matrices in fp16 (conv1) and of the BINARY patterns in fp8
    (conv2). The gcn normalization A = S*Abin*S (S = diag(1/sqrt(deg)))
    is factored out for conv2: Abin is EXACT in fp8 (entries 0/1), the
    per-node s-scales fold into the z quantization (pre-scale) and the
    PSUM->SBUF copies (post-scale), so conv2's adjacency contributes zero
    quantization error while running at fp8 DoubleRow speed (2 fp8 weights
    per PE cell, 2 contraction k-tiles per matmul, ~1.8x fp16 rate).
  - The feature embed is *replicated* (fp16): every core computes the full h
    for all 8192 nodes in node-major layout instead of all-gathering it.
  - conv1 (fp16, full precision path): zT = [A@h; A2@h].T feature-major, RAW.
  - BatchNorm is *algebraically absorbed* into the final projection:
        z_n = z*c + d  with c = gamma*rsqrt(var+eps), d = beta - mean*c
        A@z_n = (A@z)*c + rowsum(A) (x) d
    so conv2 runs on raw z and the final projection applies per-row scaled
    weights plus rank-1 corrections from host-exact rowsums.
  - z is MEAN-CENTERED (host-exact colmeans mu_z via the colsum identity
    mean(A@h) = (colsum(A)/N)@h) before fp8 quantization: the quantization
    error then scales with the per-column FLUCTUATION - exactly what BN
    normalizes by - instead of the column mean, avoiding the ~30x error
    amplification of the near-constant A2@h columns. The centering is
    algebraically exact: A@z = A@(z-mu) + rowsum(A) (x) mu, and the rank-1
    correction folds into the existing BN-absorption vector (d' = c*mu + d)
    at ZERO device cost.
  - Centered z is transposed to node-major, quantized twice (s1- and
    s2-pre-scaled fp8), and AllGathered in two feature-halves into
    addr_space="Shared" DRAM (fast HBM-HBM collective path), with
    partition-swizzled layout so gather-in and SBUF reload DMAs are
    contiguous 2KB-per-partition runs. The reloads run on the sync queue so
    they are NOT serialized behind the BN AllReduce on the gpsimd/CC queue.
    z1's gather hides under conv1's second half, z2's under conv2's first
    m-pass. BN statistics use a tiny AllReduce, off the critical path.
"""

import numpy as np

import concourse.bass as bass
import concourse.mybir as mybir
import concourse.tile as tile
from concourse import bacc
from concourse.bass_utils import run_bass_kernel_spmd
from concourse.masks import make_identity

P = 128
NCORES = 8
BN_EPS = 1e-5

F8 = mybir.dt.float8e4
F16 = mybir.dt.float16
F32 = mybir.dt.float32
DR = mybir.MatmulPerfMode.DoubleRow

FULL_CFG = dict(NT=8192, R=1024)
IN_CH = 512   # input features
H = 256       # hidden
H2 = 512      # 2*H (BN width)
O = 64        # output features
F = 7 * H     # 1792, JK concat width


def _nchunks(R):
    """Split the per-core node free-dim R into <=512 chunks (PSUM bank width)."""
    out = []
    s = 0
    while s < R:
        w = min(512, R - s)
        out.append((s, w))
        s += w
    return out


def build_program(NT, R, scales):
    """Build the SPMD Bass program. NT = total nodes, R = rows per core.
    scales = (SZ1, SZ2): power-of-2 exponents for the fp8 z quantization."""
    SZ1, SZ2 = scales
    KT = NT // P           # node k-tiles (contraction tiles)
    K2 = KT // 2           # DoubleRow k-tile pairs
    RT = R // P            # per-core node tiles (free-dim tiles / transposes)
    NCH = _nchunks(R)
    NC2 = len(NCH)
    HM = H // P            # 2  (hidden chunks)
    H2M = H2 // P          # 4
    FM = F // P            # 14
    INK = IN_CH // P       # 4

    nc = bacc.Bacc("TRN2", target_bir_lowering=False, debug=False,
                   num_devices=NCORES)

    # --- I/O -------------------------------------------------------------
    xTf = nc.dram_tensor("xTf", [IN_CH, NT], F16, kind="ExternalInput")
    xT = nc.dram_tensor("xT", [IN_CH, R], F16, kind="ExternalInput")
    abT = nc.dram_tensor("abT", [NT, R], F8, kind="ExternalInput")
    abT2 = nc.dram_tensor("abT2", [NT, R], F8, kind="ExternalInput")
    wTe = nc.dram_tensor("wTe", [IN_CH, H], F16, kind="ExternalInput")
    be = nc.dram_tensor("be", [P, HM], F32, kind="ExternalInput")
    bebc = nc.dram_tensor("bebc", [P, H], F16, kind="ExternalInput")
    wTf = nc.dram_tensor("wTf", [F, O], F16, kind="ExternalInput")
    bff = nc.dram_tensor("bff", [O, 1], F32, kind="ExternalInput")
    gam = nc.dram_tensor("gam", [P, H2M], F32, kind="ExternalInput")
    bet = nc.dram_tensor("bet", [P, H2M], F32, kind="ExternalInput")
    nmuz = nc.dram_tensor("nmuz", [P, H2M], F32, kind="ExternalInput")
    muz = nc.dram_tensor("muz", [P, H2M], F32, kind="ExternalInput")
    s1q = nc.dram_tensor("s1q", [P, RT], F32, kind="ExternalInput")
    s2q = nc.dram_tensor("s2q", [P, RT], F32, kind="ExternalInput")
    s1f = nc.dram_tensor("s1f", [P, KT], F32, kind="ExternalInput")
    s2f = nc.dram_tensor("s2f", [P, KT], F32, kind="ExternalInput")
    s1c = nc.dram_tensor("s1c", [P, R], F16, kind="ExternalInput")
    s2c = nc.dram_tensor("s2c", [P, R], F16, kind="ExternalInput")
    rsA = nc.dram_tensor("rsA", [O, R], F16, kind="ExternalInput")
    rsA2 = nc.dram_tensor("rsA2", [O, R], F16, kind="ExternalInput")
    out = nc.dram_tensor("out", [R, O], F32, kind="ExternalOutput")

    rg = [list(range(NCORES))]

    with tile.TileContext(nc) as tc:
        with (
            tc.tile_pool(name="const", bufs=1) as const,
            tc.tile_pool(name="feat", bufs=1) as feat,
            tc.tile_pool(name="tmp", bufs=2) as tmp,
            tc.tile_pool(name="stream", bufs=12) as stream,
            tc.tile_pool(name="ps", bufs=1, space="PSUM") as ps,
            tc.tile_pool(name="dram", bufs=1, space="DRAM") as dram,
        ):
            # --- CC warm-up: a tiny AllGather absorbs the collective
            # init barrier (~45us) long before the z gathers need the CC --
            wu_sb = const.tile([P, 1], F32, name="wu_sb")
            nc.vector.memset(wu_sb[:], 1.0)
            wu_in = dram.tile([P, 1], F32, name="wu_in")
            nc.gpsimd.dma_start(wu_in[:], wu_sb[:])
            wu_out = dram.tile([NCORES, P, 1], F32, name="wu_out",
                               addr_space="Shared")
            nc.gpsimd.collective_compute(
                "AllGather", mybir.AluOpType.bypass, replica_groups=rg,
                ins=[wu_in.opt()], outs=[wu_out.opt()],
            )

            # --- constants / weights (embed-critical ones first) --------
            wTe_sb = const.tile([P, INK, H], F16, name="wTe_sb")
            nc.sync.dma_start(wTe_sb[:], wTe.ap().rearrange("(k p) m -> p k m", p=P))
            bebc_sb = const.tile([P, H], F16, name="bebc_sb")
            nc.sync.dma_start(bebc_sb[:], bebc.ap())
            s1f_sb = const.tile([P, KT], F32, name="s1f_sb")
            nc.sync.dma_start(s1f_sb[:], s1f.ap())
            s2f_sb = const.tile([P, KT], F32, name="s2f_sb")
            nc.sync.dma_start(s2f_sb[:], s2f.ap())

            # full x.T, staged through a double-buffer (32 KB instead of
            # 64 KB resident) so the embed starts early and SBUF stays small
            xTf_t = xTf.ap().rearrange("(k p) n -> p k n", p=P)
            XG = 1024
            KG = XG // P

            # --- phase B1: replicated full embed, node-major ------------
            # hs_a[node, feat] = s_a[node] * relu(x @ w_embed.T + b), the
            # row-scaled h needed by the factored conv1 (A = S Abin S).
            # relu(s*x) = s*relu(x) for s>=0, so the s-scale rides the relu.
            hs1_sb = feat.tile([P, KT, H], F16, name="hs1_sb", tag="kxnA")
            hs2_sb = feat.tile([P, KT, H], F16, name="hs2_sb", tag="hs2")
            xg_sb = None
            for k in range(KT):
                if k % KG == 0:
                    g = k * P
                    xg_sb = feat.tile([P, INK, XG], F16, name=f"xg_{k}",
                                      tag="xg", bufs=2)
                    nc.sync.dma_start(xg_sb[:], xTf_t[:, :, g:g + XG])
                hps = ps.tile([P, H], F32, name=f"hps_{k}", tag=f"acc{k % 8}")
                for t in range(INK):
                    nc.tensor.matmul(
                        hps[:],
                        lhsT=xg_sb[:, t, (k % KG) * P:(k % KG + 1) * P],
                        rhs=wTe_sb[:, t, :],
                        start=(t == 0), stop=(t == INK - 1),
                    )
                nc.vector.tensor_tensor(
                    out=hps[:], in0=hps[:], in1=bebc_sb[:],
                    op=mybir.AluOpType.add)
                nc.scalar.activation(
                    hs1_sb[:, k, :], hps[:],
                    mybir.ActivationFunctionType.Relu,
                    scale=s1f_sb[:, k:k + 1])
                nc.vector.tensor_scalar(
                    out=hs2_sb[:, k, :], in0=hps[:],
                    scalar1=0.0, scalar2=s2f_sb[:, k:k + 1],
                    op0=mybir.AluOpType.max, op1=mybir.AluOpType.mult)

            # --- remaining constants (needed from B2 / conv1 onward) ----
            xT_sb = const.tile([P, INK, R], F16, name="xT_sb")
            nc.sync.dma_start(xT_sb[:], xT.ap().rearrange("(k p) n -> p k n", p=P))
            be_sb = const.tile([P, HM], F32, name="be_sb")
            nc.sync.dma_start(be_sb[:], be.ap())
            id16 = const.tile([P, P], F16, name="id16")
            make_identity(nc, id16)
            id32 = const.tile([P, P], F32, name="id32")
            make_identity(nc, id32)
            nmuz_sb = const.tile([P, H2M], F32, name="nmuz_sb")
            nc.sync.dma_start(nmuz_sb[:], nmuz.ap())
            s1q_sb = const.tile([P, RT], F32, name="s1q_sb")
            nc.sync.dma_start(s1q_sb[:], s1q.ap())
            s2q_sb = const.tile([P, RT], F32, name="s2q_sb")
            nc.sync.dma_start(s2q_sb[:], s2q.ap())
            s1c_sb = const.tile([P, R], F16, name="s1c_sb")
            nc.sync.dma_start(s1c_sb[:], s1c.ap())
            s2c_sb = const.tile([P, R], F16, name="s2c_sb")
            nc.sync.dma_start(s2c_sb[:], s2c.ap())

            # --- phase B2: local embed, feature-major (for the JK concat)
            hT_sb = feat.tile([P, HM, R], F16, name="hT_sb")
            for m in range(HM):
                for ci, (cs, cw) in enumerate(NCH):
                    eps_t = ps.tile([P, 512], F32, name=f"eps_{m}_{ci}",
                                    tag=f"acc{(m * NC2 + ci) % 8}")
                    for t in range(INK):
                        nc.tensor.matmul(
                            eps_t[:, :cw],
                            lhsT=wTe_sb[:, t, m * P:(m + 1) * P],
                            rhs=xT_sb[:, t, cs:cs + cw],
                            start=(t == 0), stop=(t == INK - 1),
                        )
                    nc.scalar.activation(
                        hT_sb[:, m, cs:cs + cw], eps_t[:, :cw],
                        mybir.ActivationFunctionType.Relu,
                        bias=be_sb[:, m:m + 1],
                    )

            # --- phase D: conv1 (factored: fp16 hs x fp8 binary Abin) ---
            # z = S (Abin @ (S h)): the streamed rhs is the EXACT binary
            # pattern in fp8 (half the bytes of fp16 normalized A, zero
            # quantization error); the output s-post-scale folds into the
            # PSUM->SBUF copy. One adjacency half at a time; each half's
            # centered transposes + dual fp8 quantize + AllGather overlap
            # the next compute.
            zT_sb = feat.tile([P, H2M, R], F16, name="zT_sb")
            zag_out = [None, None]
            for half, (src, hs_sb, sc_sb) in (
                    (0, (abT, None, None)), (1, (abT2, None, None))):
                hs_sb = hs1_sb if half == 0 else hs2_sb
                sc_sb = s1c_sb if half == 0 else s2c_sb
                zps = {}
                for m in range(HM):
                    for ci in range(NC2):
                        zps[(m, ci)] = ps.tile(
                            [P, 512], F32, name=f"zps_{half}_{m}_{ci}",
                            tag=f"acc{(half * 4 + m * NC2 + ci) % 8}")
                for k in range(KT):
                    at = stream.tile([P, R], F8, name=f"c1_{half}_{k}", tag="adj")
                    nc.sync.dma_start(at[:], src[k * P:(k + 1) * P, :])
                    for m in range(HM):
                        for ci, (cs, cw) in enumerate(NCH):
                            nc.tensor.matmul(
                                zps[(m, ci)][:, :cw],
                                lhsT=hs_sb[:, k, m * P:(m + 1) * P],
                                rhs=at[:, cs:cs + cw],
                                start=(k == 0), stop=(k == KT - 1),
                            )
                # raw z = s_out * PSUM (vector) + centered z (scalar)
                zcT = tmp.tile([P, HM, R], F16, name="zcT", tag="zcT", bufs=1)
                for m in range(HM):
                    for ci, (cs, cw) in enumerate(NCH):
                        nc.vector.tensor_tensor(
                            out=zT_sb[:, half * HM + m, cs:cs + cw],
                            in0=zps[(m, ci)][:, :cw],
                            in1=sc_sb[:, cs:cs + cw],
                            op=mybir.AluOpType.mult)
                        nc.scalar.activation(
                            zcT[:, m, cs:cs + cw],
                            zT_sb[:, half * HM + m, cs:cs + cw],
                            mybir.ActivationFunctionType.Identity,
                            bias=nmuz_sb[:, half * HM + m:half * HM + m + 1])

                # transpose centered z to node-major and quantize twice
                # (s1- and s2-pre-scaled fp8), then AllGather both
                zs_nm = tmp.tile([P, RT, 2, H], F8, name="znm", tag="znm", bufs=1)
                for fi in range(HM):
                    for nt in range(RT):
                        tps = ps.tile(
                            [P, P], F16, name=f"ztp_{half}_{fi}_{nt}",
                            tag=f"acc{(fi * RT + nt) % 4 + half * 4}")
                        nc.tensor.transpose(
                            tps[:], zcT[:, fi, nt * P:(nt + 1) * P],
                            id16[:])
                        nc.scalar.mul(
                            zs_nm[:, nt, 0, fi * P:(fi + 1) * P], tps[:],
                            s1q_sb[:, nt:nt + 1])
                        nc.scalar.mul(
                            zs_nm[:, nt, 1, fi * P:(fi + 1) * P], tps[:],
                            s2q_sb[:, nt:nt + 1])
                zin = dram.tile([P, RT, 2, H], F8, name=f"zag_in_{half}")
                nc.gpsimd.dma_start(zin[:], zs_nm[:])
                zout = dram.tile([NCORES, P, RT, 2, H], F8,
                                 name=f"zag_out_{half}", addr_space="Shared")
                nc.gpsimd.collective_compute(
                    "AllGather", mybir.AluOpType.bypass, replica_groups=rg,
                    ins=[zin.opt()], outs=[zout.opt()],
                )
                zag_out[half] = zout

            # --- gathered-z reloads: scalar queue (half 0, idle there
            # after conv1) and sync queue (half 1); emitted BEFORE phase E
            # so no BN-coefficient wait can block them. Layout [p][kt][a][f]
            # with kt=(r,nt): fully contiguous 4KB runs per (r,p).
            zfd_sb = []
            for half, eng in ((0, nc.scalar), (1, nc.sync)):
                zfd = feat.tile([P, KT, 2, H], F8, name=f"zfd_{half}",
                                tag=("hs2" if half == 0 else "kxnA"))
                eng.dma_start(
                    zfd[:].rearrange("p (r nt) a f -> p r nt a f", nt=RT),
                    zag_out[half].rearrange("r p nt a f -> p r nt a f"))
                zfd_sb.append(zfd)

            # --- phase E: BN stats + AllReduce (off critical path) ------
            stat_sb = tmp.tile([P, 2 * H2M], F32, name="stat_sb", bufs=1)
            for f in range(H2M):
                sq = tmp.tile([P, R], F16, name="sq", tag="zcT", bufs=1)
                nc.scalar.activation(
                    sq[:], zT_sb[:, f, :], mybir.ActivationFunctionType.Copy,
                    accum_out=stat_sb[:, f:f + 1])
                sq2 = tmp.tile([P, R], F16, name="sq2", tag="zcT", bufs=1)
                nc.scalar.activation(
                    sq2[:], zT_sb[:, f, :], mybir.ActivationFunctionType.Square,
                    accum_out=stat_sb[:, H2M + f:H2M + f + 1])
            ar_in = dram.tile([P, 2 * H2M], F32, name="ar_in")
            nc.gpsimd.dma_start(ar_in[:], stat_sb[:])
            ar_out = dram.tile([P, 2 * H2M], F32, name="ar_out")
            nc.gpsimd.collective_compute(
                "AllReduce", mybir.AluOpType.add, replica_groups=rg,
                ins=[ar_in.opt()], outs=[ar_out.opt()],
            )
            stat_g = tmp.tile([P, 2 * H2M], F32, name="stat_g", bufs=1)
            nc.gpsimd.dma_start(stat_g[:], ar_out[:])

            # BN coefficients c, d (feature-major [128, 4], fp32)
            gam_sb = const.tile([P, H2M], F32, name="gam_sb")
            nc.sync.dma_start(gam_sb[:], gam.ap())
            bet_sb = const.tile([P, H2M], F32, name="bet_sb")
            nc.sync.dma_start(bet_sb[:], bet.ap())
            muz_sb = const.tile([P, H2M], F32, name="muz_sb")
            nc.sync.dma_start(muz_sb[:], muz.ap())
            cmean = tmp.tile([P, H2M], F32, name="cmean", bufs=1)
            nc.scalar.mul(cmean[:], stat_g[:, 0:H2M], 1.0 / NT)
            cvar = tmp.tile([P, H2M], F32, name="cvar", bufs=1)
            nc.scalar.mul(cvar[:], stat_g[:, H2M:2 * H2M], 1.0 / NT)
            msq = tmp.tile([P, H2M], F32, name="msq", bufs=1)
            nc.vector.tensor_mul(out=msq[:], in0=cmean[:], in1=cmean[:])
            nc.vector.tensor_tensor(
                out=cvar[:], in0=cvar[:], in1=msq[:],
                op=mybir.AluOpType.subtract)
            eps_sb = tmp.tile([P, 1], F32, name="eps_sb", bufs=1)
            nc.vector.memset(eps_sb[:], BN_EPS)
            cstd = tmp.tile([P, H2M], F32, name="cstd", bufs=1)
            nc.scalar.activation(
                cstd[:], cvar[:], mybir.ActivationFunctionType.Sqrt,
                bias=eps_sb[:])
            crstd = tmp.tile([P, H2M], F32, name="crstd", bufs=1)
            nc.vector.reciprocal(crstd[:], cstd[:])
            c_t = tmp.tile([P, H2M], F32, name="c_t", bufs=1)
            nc.vector.tensor_mul(out=c_t[:], in0=crstd[:], in1=gam_sb[:])
            d_t = tmp.tile([P, H2M], F32, name="d_t", bufs=1)
            nc.vector.tensor_mul(out=d_t[:], in0=cmean[:], in1=c_t[:])
            nc.vector.tensor_tensor(
                out=d_t[:], in0=bet_sb[:], in1=d_t[:],
                op=mybir.AluOpType.subtract)
            d16 = tmp.tile([P, H2M], F16, name="d16", bufs=1)
            nc.vector.tensor_copy(out=d16[:], in_=d_t[:])
            # d' = c * mu_z + d  (U-block correction incl. the z centering)
            dp_t = tmp.tile([P, H2M], F32, name="dp_t", bufs=1)
            nc.vector.tensor_mul(out=dp_t[:], in0=c_t[:], in1=muz_sb[:])
            nc.vector.tensor_add(out=dp_t[:], in0=dp_t[:], in1=d_t[:])
            d16p = tmp.tile([P, H2M], F16, name="d16p", bufs=1)
            nc.vector.tensor_copy(out=d16p[:], in_=dp_t[:])

            # --- phase F: conv2 on centered z (fp8 DoubleRow), two ------
            # m-half passes; pass 0 (z features 0:256, from z1) only needs
            # zag_out[0], so it overlaps z2's AllGather. Each pass streams
            # both binary adjacencies.
            ab_p = abT.ap().rearrange("(k2 ko p) n -> p k2 ko n", ko=2, p=P)
            ab2_p = abT2.ap().rearrange("(k2 ko p) n -> p k2 ko n", ko=2, p=P)
            u_sb = feat.tile([P, 2 * H2M, R], F16, name="u_sb")
            for half in (0, 1):
                zfd = zfd_sb[half]
                ups = {}
                for a in (0, 1):
                    for m in range(HM):
                        for ci in range(NC2):
                            ups[(a, m, ci)] = ps.tile(
                                [P, 512], F32, name=f"ups_{half}_{a}_{m}_{ci}",
                                tag=f"acc{(a * 4 + m * NC2 + ci) % 8}")
                for k2 in range(K2):
                    at = stream.tile([P, 2, R], F8, name=f"c2a_{half}_{k2}",
                                     tag="adj")
                    nc.sync.dma_start(at[:], ab_p[:, k2])
                    at2 = stream.tile([P, 2, R], F8, name=f"c2b_{half}_{k2}",
                                      tag="adj")
                    nc.sync.dma_start(at2[:], ab2_p[:, k2])
                    for m in range(HM):
                        for ci, (cs, cw) in enumerate(NCH):
                            nc.tensor.matmul(
                                ups[(0, m, ci)][:, :cw],
                                lhsT=zfd[:, 2 * k2:2 * k2 + 2, 0,
                                         m * P:(m + 1) * P],
                                rhs=at[:, :, cs:cs + cw],
                                start=(k2 == 0), stop=(k2 == K2 - 1),
                                perf_mode=DR,
                            )
                            nc.tensor.matmul(
                                ups[(1, m, ci)][:, :cw],
                                lhsT=zfd[:, 2 * k2:2 * k2 + 2, 1,
                                         m * P:(m + 1) * P],
                                rhs=at2[:, :, cs:cs + cw],
                                start=(k2 == 0), stop=(k2 == K2 - 1),
                                perf_mode=DR,
                            )
                # u feature layout: chunks 0..3 = U1 (A@zc), 4..7 = U2
                # (A2@zc); this pass produces z-feature chunks {half*2,
                # half*2+1} of each. Post-scale s_a * 2^-SZa (DVE).
                for a, (su, eng) in ((0, (s1c_sb, nc.vector)),
                                     (1, (s2c_sb, nc.vector))):
                    for m in range(HM):
                        for ci, (cs, cw) in enumerate(NCH):
                            eng.tensor_tensor(
                                out=u_sb[:, a * H2M + half * HM + m, cs:cs + cw],
                                in0=ups[(a, m, ci)][:, :cw],
                                in1=su[:, cs:cs + cw],
                                op=mybir.AluOpType.mult)

            # --- phase G: final projection with absorbed BN -------------
            wTf_sb = const.tile([P, FM, O], F16, name="wTf_sb")
            nc.sync.dma_start(wTf_sb[:], wTf.ap().rearrange("(k p) m -> p k m", p=P))
            bff_sb = const.tile([O, 1], F32, name="bff_sb")
            nc.sync.dma_start(bff_sb[:], bff.ap())
            rsA_sb = const.tile([O, R], F16, name="rsA_sb")
            nc.sync.dma_start(rsA_sb[:], rsA.ap())
            rsA2_sb = const.tile([O, R], F16, name="rsA2_sb")
            nc.sync.dma_start(rsA2_sb[:], rsA2.ap())

            # s_0 = W_zn @ d ; s_j = W_Uj @ d' (j=1,2), from UNSCALED wTf
            s_cols = tmp.tile([O, 3], F32, name="s_cols", bufs=1)
            for j, (base, dv) in enumerate((
                    (HM, d16), (HM + H2M, d16p), (HM + 2 * H2M, d16p))):
                sps = ps.tile([O, 1], F32, name=f"sps_{j}", tag=f"acc{j}")
                for t in range(H2M):
                    nc.tensor.matmul(
                        sps[:], lhsT=wTf_sb[:, base + t, :],
                        rhs=dv[:, t:t + 1],
                        start=(t == 0), stop=(t == H2M - 1))
                nc.vector.tensor_copy(out=s_cols[:, j:j + 1], in_=sps[:])
            s0b = tmp.tile([O, 1], F32, name="s0b", bufs=1)
            nc.vector.tensor_add(out=s0b[:], in0=s_cols[:, 0:1], in1=bff_sb[:])

            # scale wTf rows in place: z_n block by c, U blocks by
            # c * 2^-SZa (u_sb is stored 2^SZa-scaled; descale folds here)
            cu1_t = tmp.tile([P, H2M], F32, name="cu1_t", bufs=1)
            nc.scalar.mul(cu1_t[:], c_t[:], float(2.0 ** -SZ1))
            cu2_t = tmp.tile([P, H2M], F32, name="cu2_t", bufs=1)
            nc.scalar.mul(cu2_t[:], c_t[:], float(2.0 ** -SZ2))
            for t in range(HM, FM):
                ch = (t - HM) % H2M
                cv = c_t if t < HM + H2M else (
                    cu1_t if t < HM + 2 * H2M else cu2_t)
                nc.vector.tensor_scalar_mul(
                    wTf_sb[:, t, :], wTf_sb[:, t, :], cv[:, ch:ch + 1])

            # outT[64, R] = wTf'.T @ jkT + (s0+bf) + s1 (x) rsA + s2 (x) rsA2
            def jk_rhs(t):
                if t < HM:
                    return hT_sb[:, t, :]
                if t < HM + H2M:
                    return zT_sb[:, t - HM, :]
                return u_sb[:, t - HM - H2M, :]

            outsb = tmp.tile([O, R], F32, name="outsb", bufs=1)
            o_nm = tmp.tile([P, RT, O], F32, name="o_nm", bufs=1)
            NTC = 512 // P
            for ci, (cs, cw) in enumerate(NCH):
                ops = ps.tile([O, 512], F32, name=f"ops_{ci}", tag=f"acc{4 + ci}")
                for t in range(FM):
                    nc.tensor.matmul(
                        ops[:, :cw], lhsT=wTf_sb[:, t, :],
                        rhs=jk_rhs(t)[:, cs:cs + cw],
                        start=(t == 0), stop=(t == FM - 1))
                nc.vector.tensor_scalar_add(
                    outsb[:, cs:cs + cw], ops[:, :cw], s0b[:])
                rk1 = tmp.tile([O, 512], F32, name=f"rk_{ci}", tag="rk", bufs=2)
                nc.vector.tensor_scalar_mul(
                    rk1[:, :cw], rsA_sb[:, cs:cs + cw], s_cols[:, 1:2])
                nc.vector.tensor_add(
                    out=outsb[:, cs:cs + cw], in0=outsb[:, cs:cs + cw],
                    in1=rk1[:, :cw])
                rk2 = tmp.tile([O, 512], F32, name=f"rk2_{ci}", tag="rk", bufs=2)
                nc.vector.tensor_scalar_mul(
                    rk2[:, :cw], rsA2_sb[:, cs:cs + cw], s_cols[:, 2:3])
                nc.vector.tensor_add(
                    out=outsb[:, cs:cs + cw], in0=outsb[:, cs:cs + cw],
                    in1=rk2[:, :cw])
                # transpose this chunk's node-tiles immediately
                for ntl in range(NTC):
                    nt = ci * NTC + ntl
                    tps32 = ps.tile([P, O], F32, name=f"otp_{nt}",
                                    tag=f"acc{nt % 4}")
                    nc.tensor.transpose(
                        tps32[:], outsb[:, nt * P:(nt + 1) * P], id32[:O, :O])
                    nc.any.tensor_copy(out=o_nm[:, nt, :], in_=tps32[:])
            nc.sync.dma_start(
                out.ap().rearrange("(nt p) o -> p nt o", p=P), o_nm[:])

    nc.compile()
    return nc


_PROGRAM_CACHE = {}


def _get_program(NT, R, scales):
    key = (NT, R, scales)
    if key not in _PROGRAM_CACHE:
        _PROGRAM_CACHE[key] = build_program(NT, R, scales)
    return _PROGRAM_CACHE[key]


def _p2(cap, v):
    """Largest power-of-2 exponent s with v * 2^s <= cap."""
    return int(np.floor(np.log2(cap / max(float(v), 1e-30))))


def _prep(inputs):
    """Host-side shared prep: exact h, z column means, s-scales, exponents."""
    x = np.asarray(inputs["x"], np.float32)
    we = np.asarray(inputs["w_embed"], np.float32)
    be = np.asarray(inputs["b_embed"], np.float32)
    adj = np.asarray(inputs["adj_t"], np.float32)
    adj2 = np.asarray(inputs["adj_t2"], np.float32)
    NT = x.shape[0]
    h = np.maximum(x @ we.T + be, 0)
    # exact per-feature means of z = [A@h; A2@h] via the colsum identity
    muz = np.concatenate([(adj.sum(0) / NT) @ h, (adj2.sum(0) / NT) @ h])
    d1 = (adj > 0).sum(1).astype(np.float32)
    d2 = (adj2 > 0).sum(1).astype(np.float32)
    s1 = np.where(d1 > 0, 1.0 / np.sqrt(np.maximum(d1, 1e-12)), 0.0).astype(np.float32)
    s2 = np.where(d2 > 0, 1.0 / np.sqrt(np.maximum(d2, 1e-12)), 0.0).astype(np.float32)
    # safe bound for |z - muz|
    hmax = float(np.abs(h).max())
    zb = float(max(adj.sum(1).max(), adj2.sum(1).max())) * hmax \
        + float(np.abs(muz).max()) + 1e-6
    SZ1 = _p2(192.0, s1.max() * zb)
    SZ2 = _p2(192.0, s2.max() * zb)
    return dict(h=h, muz=muz, s1=s1, s2=s2, SZ1=SZ1, SZ2=SZ2)


def compute_scales(inputs, prep=None):
    prep = prep if prep is not None else _prep(inputs)
    return (prep["SZ1"], prep["SZ2"])


def make_in_maps(inputs, NT, R, scales, prep=None):
    """Shard full inputs into per-core input maps (host-side, numpy)."""
    SZ1, SZ2 = scales
    prep = prep if prep is not None else _prep(inputs)
    muz, s1, s2 = prep["muz"], prep["s1"], prep["s2"]
    F8NP = mybir.dt.np(F8)
    x = np.asarray(inputs["x"], np.float32)
    adj = np.asarray(inputs["adj_t"], np.float32)
    adj2 = np.asarray(inputs["adj_t2"], np.float32)
    we = np.asarray(inputs["w_embed"], np.float32)
    be = np.asarray(inputs["b_embed"], np.float32)
    gam = np.asarray(inputs["bn_gamma"], np.float32)
    bet = np.asarray(inputs["bn_beta"], np.float32)
    wf = np.asarray(inputs["w_fin"], np.float32)
    bf = np.asarray(inputs["b_fin"], np.float32)

    H2M = H2 // P
    RT = R // P
    xTf_h = np.ascontiguousarray(x.T).astype(np.float16)
    wTe_h = np.ascontiguousarray(we.T).astype(np.float16)
    be_h = np.ascontiguousarray(be.reshape(H // P, P).T).astype(np.float32)
    bebc_h = np.ascontiguousarray(
        np.broadcast_to(be[None, :], (P, H))).astype(np.float16)
    wTf_h = np.ascontiguousarray(wf.T).astype(np.float16)
    bff_h = np.ascontiguousarray(bf[:, None]).astype(np.float32)
    gam_h = np.ascontiguousarray(gam.reshape(H2M, P).T).astype(np.float32)
    bet_h = np.ascontiguousarray(bet.reshape(H2M, P).T).astype(np.float32)
    muz_h = np.ascontiguousarray(muz.reshape(H2M, P).T).astype(np.float32)
    nmuz_h = np.ascontiguousarray(-muz.reshape(H2M, P).T).astype(np.float32)

    KT = NT // P
    s1f_h = np.ascontiguousarray(s1.reshape(KT, P).T).astype(np.float32)
    s2f_h = np.ascontiguousarray(s2.reshape(KT, P).T).astype(np.float32)
    in_maps = []
    for r in range(NCORES):
        rows = slice(r * R, (r + 1) * R)
        rsA_h = np.ascontiguousarray(
            np.broadcast_to(adj[rows].sum(1)[None, :], (O, R))).astype(np.float16)
        rsA2_h = np.ascontiguousarray(
            np.broadcast_to(adj2[rows].sum(1)[None, :], (O, R))).astype(np.float16)
        s1r, s2r = s1[rows], s2[rows]
        in_maps.append({
            "xTf": xTf_h,
            "xT": np.ascontiguousarray(x[rows].T).astype(np.float16),
            "abT": np.ascontiguousarray(
                (adj[rows] > 0).T.astype(np.float32)).astype(F8NP),
            "abT2": np.ascontiguousarray(
                (adj2[rows] > 0).T.astype(np.float32)).astype(F8NP),
            "wTe": wTe_h, "be": be_h, "bebc": bebc_h, "wTf": wTf_h,
            "bff": bff_h, "gam": gam_h, "bet": bet_h,
            "nmuz": nmuz_h, "muz": muz_h,
            "s1q": np.ascontiguousarray(
                (s1r * 2.0 ** SZ1).reshape(RT, P).T).astype(np.float32),
            "s2q": np.ascontiguousarray(
                (s2r * 2.0 ** SZ2).reshape(RT, P).T).astype(np.float32),
            "s1f": s1f_h, "s2f": s2f_h,
            "s1c": np.ascontiguousarray(np.broadcast_to(
                s1r[None, :], (P, R))).astype(np.float16),
            "s2c": np.ascontiguousarray(np.broadcast_to(
                s2r[None, :], (P, R))).astype(np.float16),
            "rsA": rsA_h, "rsA2": rsA2_h,
        })
    return in_maps


def kernel(**inputs):
    NT, R = FULL_CFG["NT"], FULL_CFG["R"]
    prep = _prep(inputs)
    scales = compute_scales(inputs, prep)
    nc = _get_program(NT, R, scales)
    in_maps = make_in_maps(inputs, NT, R, scales, prep)
    res = run_bass_kernel_spmd(nc, in_maps, core_ids=list(range(NCORES)))
    out = np.concatenate(
        [res.results[r]["out"] for r in range(NCORES)], axis=0)
    return out.astype(np.float32)
